# revision 1
# baseline (speedup 1.0000x reference)
"""CMAttention Trainium2 kernel (8-core SPMD).

Reference computation (per nn_CMAttention):
  q_x = (x @ Wq_x.T)  -> [b, 16, n, 64],  q_a likewise
  kv_x = x @ Wkv_x.T -> k_x, v_x [b, 1, n, 64] (single shared KV head), kv_a likewise
  l2norm + learned scales on q_x/q_a (per head) and k_x/k_a (shared)
  q = concat(q_x, q_a) [b,16,n,128]; k, v likewise [b,1,n,128]
  rotary(q, k) over the 128-dim concat axis; SDPA with softmax over keys.

Sharding: heads are split across the 8 cores (2 heads per core, both batches);
the shared KV projection is computed replicated on every core.

Device-side layout trick: everything is computed "transposed" (feature dim on
partitions, sequence on the free axis), so the host passes x/a pre-transposed
and reassembles the output. Softmax runs on S^T tiles: the row-sum over keys
is a ones-matmul partition reduction accumulated in PSUM; no max subtraction
is needed because q/k rows are l2-normalized (|scores*scale| <= ~0.2 for
unit scales; fp32 exp is safe far beyond that).
"""

import numpy as np
from contextlib import ExitStack

import concourse.bass as bass
from concourse import bacc
import concourse.mybir as mybir
import concourse.tile as tile
from concourse.masks import make_identity

F32 = mybir.dt.float32
AF = mybir.ActivationFunctionType
ALU = mybir.AluOpType

P = 128
B, N, DIM = 2, 2048, 1024
HEADS, DH, ROT = 16, 64, 128
NCORES, HPC = 8, 2          # 2 heads per core
KT = DIM // P               # 8 contraction tiles
SM_SCALE = float(1.0 / np.sqrt(ROT))


def build_nc(n=N, nb=B, stage=4):
    CH = min(512, n)        # matmul moving-operand chunk (fp32 max 512)
    NCH = n // CH
    SU = min(1024, n)       # attention superunit width (2 PSUM banks)
    NSU = n // SU
    SUC = SU // CH
    NJT = n // P            # key tiles

    nc = bacc.Bacc()
    dp = nc.declare_dram_parameter
    xT = dp("xT", [nb, DIM, n], F32, isOutput=False)
    aT = dp("aT", [nb, DIM, n], F32, isOutput=False)
    wqx = dp("wqx", [DIM, P], F32, isOutput=False)
    wqa = dp("wqa", [DIM, P], F32, isOutput=False)
    wkvx = dp("wkvx", [DIM, P], F32, isOutput=False)   # cols [k_x | v_x]
    wkva = dp("wkva", [DIM, P], F32, isOutput=False)   # cols [v_a | k_a] (host-permuted)
    sqx = dp("sqx", [P, 1], F32, isOutput=False)
    sqa = dp("sqa", [P, 1], F32, isOutput=False)
    sk = dp("sk", [P, 1], F32, isOutput=False)         # rows 0:64 kx_scale, 64:128 ka_scale
    cosT = dp("cosT", [P, n], F32, isOutput=False)     # [cos64; cos64]
    sinT = dp("sinT", [P, n], F32, isOutput=False)     # [-sin64; sin64]
    out = dp("out", [nb, HPC, ROT, n], F32, isOutput=True)

    with ExitStack() as ctx:
        tc = ctx.enter_context(tile.TileContext(nc))
        consts = ctx.enter_context(tc.tile_pool(name="consts", bufs=1))
        sb = ctx.enter_context(tc.tile_pool(name="sb", bufs=1))

        ones = consts.tile([P, P], F32)
        nc.vector.memset(ones, 1.0)
        eps_sb = consts.tile([P, 1], F32)
        nc.vector.memset(eps_sb, 1e-24)
        ident = consts.tile([P, P], F32)
        make_identity(nc, ident)

        sqx_sb = consts.tile([P, 1], F32)
        nc.gpsimd.dma_start(out=sqx_sb, in_=sqx[:])
        sqa_sb = consts.tile([P, 1], F32)
        nc.gpsimd.dma_start(out=sqa_sb, in_=sqa[:])
        sk_sb = consts.tile([P, 1], F32)
        nc.gpsimd.dma_start(out=sk_sb, in_=sk[:])
        cos_sb = consts.tile([P, n], F32)
        nc.gpsimd.dma_start(out=cos_sb, in_=cosT[:])
        sin_sb = consts.tile([P, n], F32)
        nc.gpsimd.dma_start(out=sin_sb, in_=sinT[:])

        w_sb = {}
        for name, hdl in (("wqx", wqx), ("wqa", wqa), ("wkvx", wkvx), ("wkva", wkva)):
            t = consts.tile([P, KT, P], F32, name=f"w_{name}")
            nc.gpsimd.dma_start(out=t, in_=hdl[:].rearrange("(kt p) m -> p kt m", p=P))
            w_sb[name] = t

        for b in range(nb):
            # ---------------- projections ----------------
            # QX/QA rows: [h0 dims | h1 dims]; KVX rows [k_x | v_x]; KVA rows [v_a | k_a]
            QX = sb.tile([P, n], F32, tag="big", bufs=8)
            QA = sb.tile([P, n], F32, tag="big", bufs=8)
            KVX = sb.tile([P, n], F32, tag="big", bufs=8)
            KVA = sb.tile([P, n], F32, tag="big", bufs=8)
            with tc.tile_pool(name=f"pj{b}", bufs=1, space="PSUM") as pj:
                for src, wq_t, wkv_t, qdst, kvdst in (
                    (xT, w_sb["wqx"], w_sb["wkvx"], QX, KVX),
                    (aT, w_sb["wqa"], w_sb["wkva"], QA, KVA),
                ):
                    psQ = [pj.tile([P, CH], F32, tag=f"pq{c}", name=f"psq{c}")
                           for c in range(NCH)]
                    psK = [pj.tile([P, CH], F32, tag=f"pk{c}", name=f"psk{c}")
                           for c in range(NCH)]
                    for kg in range(2):
                        kts = []
                        for i in range(KT // 2):
                            ki = kg * (KT // 2) + i
                            t = sb.tile([P, n], F32, tag="ktile", bufs=5)
                            nc.sync.dma_start(out=t, in_=src[b, ki * P:(ki + 1) * P, :])
                            kts.append((ki, t))
                        for c in range(NCH):
                            cs = slice(c * CH, (c + 1) * CH)
                            for ki, t in kts:
                                nc.tensor.matmul(psQ[c], wq_t[:, ki, :], t[:, cs],
                                                 start=(ki == 0), stop=(ki == KT - 1))
                                nc.tensor.matmul(psK[c], wkv_t[:, ki, :], t[:, cs],
                                                 start=(ki == 0), stop=(ki == KT - 1))
                    for c in range(NCH):
                        cs = slice(c * CH, (c + 1) * CH)
                        nc.vector.tensor_copy(qdst[:, cs], psQ[c])
                        nc.vector.tensor_copy(kvdst[:, cs], psK[c])

            if stage == 1:
                nc.sync.dma_start(out=out[b, 0], in_=QX)
                nc.sync.dma_start(out=out[b, 1], in_=QA)
                continue

            # ---------------- V transpose ----------------
            # V_jt [j, d]: cols 0:64 = v_x (KVX rows 64:128), cols 64:128 = v_a (KVA rows 0:64)
            V = []
            if stage != 17:
                with tc.tile_pool(name=f"vt{b}", bufs=2, space="PSUM") as vtp:
                    for jt in range(NJT):
                        js = slice(jt * P, (jt + 1) * P)
                        # matmul PSUM outputs must start bank-aligned: one
                        # tile (= one bank) per transposed half.
                        psv1 = vtp.tile([P, DH], F32, tag="v1")
                        psv2 = vtp.tile([P, DH], F32, tag="v2")
                        nc.tensor.transpose(psv1, KVX[DH:P, js], ident[DH:P, DH:P])
                        nc.tensor.transpose(psv2, KVA[0:DH, js], ident[0:DH, 0:DH])
                        vj = sb.tile([P, P], F32, tag="vsb", bufs=NJT)
                        nc.vector.tensor_copy(vj[:, 0:DH], psv1)
                        nc.vector.tensor_copy(vj[:, DH:P], psv2)
                        V.append(vj)
            if stage == 15:
                nc.sync.dma_start(out=out[b, 0, :, 0:P], in_=V[0])
                nc.sync.dma_start(out=out[b, 0, :, P:2 * P], in_=V[1])
                nc.sync.dma_start(out=out[b, 1], in_=QX)
                continue

            # ---------------- qk-norm ----------------
            QXn = sb.tile([P, n], F32, tag="big", bufs=8)
            QAn = sb.tile([P, n], F32, tag="big", bufs=8)
            KN = sb.tile([P, n], F32, tag="big", bufs=8)
            with tc.tile_pool(name=f"nm{b}", bufs=2, space="PSUM") as nm:
                # (src, dst, scale, list of 64-row ranges)
                streams = (
                    (QX, QXn, sqx_sb, (0, P)),
                    (QA, QAn, sqa_sb, (0, P)),
                    (KVX, KN, sk_sb, (0, DH)),
                    (KVA, KN, sk_sb, (DH, P)),
                )
                for src, dst, sc, (r0, r1) in streams:
                    q2 = sb.tile([P, n], F32, tag="q2", bufs=1)
                    rcp = sb.tile([P, n], F32, tag="rcp", bufs=1)
                    nc.vector.tensor_mul(q2[r0:r1, :], src[r0:r1, :], src[r0:r1, :])
                    for c in range(NCH):
                        cs = slice(c * CH, (c + 1) * CH)
                        psr = nm.tile([P, CH], F32, tag="r")
                        for h0 in range(r0, r1, DH):
                            h1 = h0 + DH
                            nc.tensor.matmul(psr[h0:h1, :], ones[h0:h1, 0:DH],
                                             q2[h0:h1, cs], start=True, stop=True)
                        nc.scalar.activation(psr[r0:r1, :], psr[r0:r1, :], AF.Sqrt,
                                             bias=eps_sb[r0:r1, :], scale=1.0)
                        nc.vector.reciprocal(rcp[r0:r1, cs], psr[r0:r1, :])
                    # dst = (src * scale) * (1/norm)
                    nc.vector.scalar_tensor_tensor(
                        dst[r0:r1, :], src[r0:r1, :], sc[r0:r1, :], rcp[r0:r1, :],
                        op0=ALU.mult, op1=ALU.mult)

            # ---------------- rotary ----------------
            # rot(t)[0:64] = t[0:64]*cos64 - t[64:128]*sin64
            # rot(t)[64:128] = t[64:128]*cos64 + t[0:64]*sin64
            # sin_sb carries the sign: rows 0:64 = -sin64, rows 64:128 = +sin64.
            if stage in (2, 17):
                nc.sync.dma_start(out=out[b, 0], in_=QXn)
                nc.sync.dma_start(out=out[b, 1], in_=KN)
                continue

            # Walrus requires identical start partitions for all DVE operands,
            # so the half-swapped companions are built with SBUF->SBUF DMAs.
            QH, QHsw = [], []
            for h in range(HPC):
                hs = slice(h * DH, (h + 1) * DH)
                qh = sb.tile([P, n], F32, tag="big", bufs=8, name=f"qh{h}")
                nc.sync.dma_start(out=qh[0:DH, :], in_=QXn[hs, :])
                nc.sync.dma_start(out=qh[DH:P, :], in_=QAn[hs, :])
                qsw = sb.tile([P, n], F32, tag="big", bufs=8, name=f"qsw{h}")
                nc.sync.dma_start(out=qsw[0:DH, :], in_=QAn[hs, :])
                nc.sync.dma_start(out=qsw[DH:P, :], in_=QXn[hs, :])
                QH.append(qh)
                QHsw.append(qsw)
            KHsw = sb.tile([P, n], F32, tag="big", bufs=8)
            nc.sync.dma_start(out=KHsw[0:DH, :], in_=KN[DH:P, :])
            nc.sync.dma_start(out=KHsw[DH:P, :], in_=KN[0:DH, :])

            # q_rot = qh*cos + qh_swapped*sin_signed ; result overwrites qh/KN
            qrot = []
            for h in range(HPC):
                tcos = sb.tile([P, n], F32, tag="tcos", bufs=1)
                tsin = sb.tile([P, n], F32, tag="tsin", bufs=1)
                nc.vector.tensor_mul(tcos, QH[h], cos_sb)
                nc.vector.tensor_mul(tsin, QHsw[h], sin_sb)
                nc.vector.tensor_add(QH[h], tcos, tsin)
                qrot.append(QH[h])
            tcos = sb.tile([P, n], F32, tag="tcos", bufs=1)
            tsin = sb.tile([P, n], F32, tag="tsin", bufs=1)
            nc.vector.tensor_mul(tcos, KN, cos_sb)
            nc.vector.tensor_mul(tsin, KHsw, sin_sb)
            krot = KN
            nc.vector.tensor_add(krot, tcos, tsin)

            if stage == 3:
                nc.sync.dma_start(out=out[b, 0], in_=qrot[0])
                nc.sync.dma_start(out=out[b, 1], in_=krot)
                continue

            # ---------------- attention ----------------
            with tc.tile_pool(name=f"at{b}", bufs=1, space="PSUM") as at:
                for h in range(HPC):
                    qr = qrot[h]
                    for su in range(NSU):
                        ps_o = at.tile([P, SU], F32, tag="o", bufs=1)
                        ps_e = at.tile([P, SU], F32, tag="e", bufs=1)
                        for jt in range(NJT):
                            js = slice(jt * P, (jt + 1) * P)
                            ps_s = at.tile([P, SU], F32, tag="s", bufs=2)
                            for cc in range(SUC):
                                el = slice(cc * CH, (cc + 1) * CH)
                                il = slice(su * SU + cc * CH, su * SU + (cc + 1) * CH)
                                nc.tensor.matmul(ps_s[:, el], krot[:, js], qr[:, il],
                                                 start=True, stop=True)
                            es = sb.tile([P, SU], F32, tag="es", bufs=2)
                            nc.scalar.activation(es, ps_s, AF.Exp, bias=0.0,
                                                 scale=SM_SCALE)
                            for cc in range(SUC):
                                el = slice(cc * CH, (cc + 1) * CH)
                                nc.tensor.matmul(ps_e[:, el], ones, es[:, el],
                                                 start=(jt == 0), stop=(jt == NJT - 1))
                                nc.tensor.matmul(ps_o[:, el], V[jt], es[:, el],
                                                 start=(jt == 0), stop=(jt == NJT - 1))
                        rec = sb.tile([P, SU], F32, tag="rec", bufs=2)
                        nc.vector.reciprocal(rec, ps_e)
                        on = sb.tile([P, SU], F32, tag="on", bufs=2)
                        nc.vector.tensor_mul(on, ps_o, rec)
                        nc.sync.dma_start(out=out[b, h, :, su * SU:(su + 1) * SU],
                                          in_=on)
    nc.finalize()
    return nc


# ---------------------------------------------------------------------------
# host side
# ---------------------------------------------------------------------------

_NC_CACHE = {}


def get_nc(n=N, nb=B):
    key = (n, nb)
    if key not in _NC_CACHE:
        _NC_CACHE[key] = build_nc(n, nb)
    return _NC_CACHE[key]


def rotary_tables(n):
    inv_freq = 1.0 / (10000.0 ** (np.arange(0, ROT, 2, dtype=np.float64) / ROT))
    freqs = np.outer(np.arange(n, dtype=np.float64), inv_freq)  # [n, 64]
    cos64 = np.cos(freqs).T.astype(np.float32)                  # [64, n]
    sin64 = np.sin(freqs).T.astype(np.float32)
    cosT = np.ascontiguousarray(np.concatenate([cos64, cos64], 0))
    sinT = np.ascontiguousarray(np.concatenate([-sin64, sin64], 0))
    return cosT, sinT


def prep_in_maps(inputs, n=N, nb=B, ncores=NCORES):
    g = {k: np.ascontiguousarray(np.asarray(v, dtype=np.float32))
         for k, v in inputs.items()}
    xT = np.ascontiguousarray(g["x"].transpose(0, 2, 1))
    aT = np.ascontiguousarray(g["a"].transpose(0, 2, 1))
    wkvx = np.ascontiguousarray(g["Wkv_x"].T)                      # cols [kx|vx]
    wkva = np.ascontiguousarray(
        np.concatenate([g["Wkv_a"][DH:2 * DH], g["Wkv_a"][0:DH]], 0).T)  # cols [va|ka]
    sk = np.ascontiguousarray(
        np.concatenate([g["kx_scale"][0, 0], g["ka_scale"][0, 0]])[:, None])
    cosT, sinT = rotary_tables(n)

    shared = dict(xT=xT, aT=aT, wkvx=wkvx, wkva=wkva, sk=sk, cosT=cosT, sinT=sinT)
    in_maps = []
    for c in range(ncores):
        h0 = c * HPC
        m = dict(shared)
        m["wqx"] = np.ascontiguousarray(g["Wq_x"][h0 * DH:(h0 + HPC) * DH].T)
        m["wqa"] = np.ascontiguousarray(g["Wq_a"][h0 * DH:(h0 + HPC) * DH].T)
        m["sqx"] = np.ascontiguousarray(
            np.concatenate([g["qx_scale"][h0 + i, 0] for i in range(HPC)])[:, None])
        m["sqa"] = np.ascontiguousarray(
            np.concatenate([g["qa_scale"][h0 + i, 0] for i in range(HPC)])[:, None])
        in_maps.append(m)
    return in_maps


def gather_out(results, n=N, nb=B, ncores=NCORES):
    full = np.empty((nb, n, HEADS * ROT), np.float32)
    for c in range(ncores):
        o = np.asarray(results[c]["out"])          # [nb, HPC, ROT, n]
        for h in range(HPC):
            gh = c * HPC + h
            full[:, :, gh * ROT:(gh + 1) * ROT] = o[:, h].transpose(0, 2, 1)
    return full


def kernel(**inputs):
    from concourse.bass_utils import run_bass_kernel_spmd
    nc = get_nc(N, B)
    in_maps = prep_in_maps(inputs, N, B, NCORES)
    res = run_bass_kernel_spmd(nc, in_maps, list(range(NCORES)))
    return gather_out(res.results, N, B, NCORES)


if __name__ == "__main__":
    nc = build_nc(256, 1)
    print("build ok")



# revision 8
# speedup vs baseline: 2.3205x; 2.3205x over previous
"""CMAttention Trainium2 kernel (8-core SPMD, bf16 compute).

Reference computation (per nn_CMAttention):
  q_x = (x @ Wq_x.T)  -> [b, 16, n, 64],  q_a likewise
  kv_x = x @ Wkv_x.T -> k_x, v_x [b, 1, n, 64] (single shared KV head), kv_a likewise
  l2norm + learned scales on q_x/q_a (per head) and k_x/k_a (shared)
  q = concat(q_x, q_a) [b,16,n,128]; k, v likewise [b,1,n,128]
  rotary(q, k) over the 128-dim concat axis; SDPA with softmax over keys.

Sharding: each core owns ONE batch (core//4) and FOUR heads ((core%4)*4 ..).
The shared KV projection is computed replicated on the 4 cores of a batch.

Device-side layout: everything is computed "transposed" (feature dim on
partitions, sequence on the free axis). All matmuls run in bf16 (fp32 matmul
is 4 cycles/column on TRN2; bf16 is 1), accumulating in fp32 PSUM. Softmax
runs on S^T tiles: the row-sum over keys is a ones-matmul partition reduction
accumulated in PSUM; no max subtraction is needed because q/k rows are
l2-normalized (|scores*scale| <= ~0.2 for unit scales).

Per-head rotary layout trick: the qk-norm scalar_tensor_tensor writes its
output DIRECTLY into the per-head [x-half; a-half] rotary tiles wherever the
partition ranges line up (DVE ops need matching start partitions); the
mismatched half of each tile is filled with one SBUF->SBUF DMA from its
companion tile.
"""

import numpy as np
import ml_dtypes
from contextlib import ExitStack

import concourse.bass as bass
from concourse import bacc
import concourse.mybir as mybir
import concourse.tile as tile
from concourse.masks import make_identity

F32 = mybir.dt.float32
BF16 = mybir.dt.bfloat16
AF = mybir.ActivationFunctionType
ALU = mybir.AluOpType
NPBF = ml_dtypes.bfloat16

P = 128
B, N, DIM = 2, 2048, 1024
HEADS, DH, ROT = 16, 64, 128
NCORES = 8
HPC = 4                     # heads per core (one batch per core)
KT = DIM // P               # 8 contraction tiles
SM_SCALE = float(1.0 / np.sqrt(ROT))


def build_nc(n=N, stage=0):
    CH = min(512, n)        # fp32 PSUM bank = 512 floats
    NCH = n // CH
    SU = min(1024, n)       # attention superunit width (2 PSUM banks)
    NSU = n // SU
    SUC = SU // CH
    NJT = n // P            # key tiles

    nc = bacc.Bacc()
    dp = nc.declare_dram_parameter
    xT = dp("xT", [DIM, n], BF16, isOutput=False)
    aT = dp("aT", [DIM, n], BF16, isOutput=False)
    wqx = dp("wqx", [DIM, HPC * DH], BF16, isOutput=False)
    wqa = dp("wqa", [DIM, HPC * DH], BF16, isOutput=False)
    wkvx = dp("wkvx", [DIM, P], BF16, isOutput=False)  # cols [k_x | v_x]
    wkva = dp("wkva", [DIM, P], BF16, isOutput=False)  # cols [v_a | k_a] (host-permuted)
    sqx = dp("sqx", [P, 2], F32, isOutput=False)       # col t: heads (2t, 2t+1)
    sqa = dp("sqa", [P, 2], F32, isOutput=False)
    sk = dp("sk", [P, 1], F32, isOutput=False)         # rows 0:64 kx_scale, 64:128 ka_scale
    cosT = dp("cosT", [P, n], BF16, isOutput=False)    # [cos64; cos64]
    sinT = dp("sinT", [P, n], BF16, isOutput=False)    # [-sin64; sin64]
    out = dp("out", [HPC, ROT, n], BF16, isOutput=True)

    with ExitStack() as ctx:
        tc = ctx.enter_context(tile.TileContext(nc))
        consts = ctx.enter_context(tc.tile_pool(name="consts", bufs=1))
        sb = ctx.enter_context(tc.tile_pool(name="sb", bufs=1))

        ones = consts.tile([P, P], BF16)
        nc.vector.memset(ones, 1.0)
        # block-diagonal ones: per-64-group partition sums in one matmul
        bdiag = consts.tile([P, P], BF16)
        nc.vector.memset(bdiag, 0.0)
        nc.vector.memset(bdiag[0:DH, 0:DH], 1.0)
        nc.vector.memset(bdiag[DH:P, DH:P], 1.0)
        eps_sb = consts.tile([P, 1], F32)
        nc.vector.memset(eps_sb, 1e-24)
        ident = consts.tile([P, P], BF16)
        make_identity(nc, ident)

        sqx_sb = consts.tile([P, 2], F32)
        nc.gpsimd.dma_start(out=sqx_sb, in_=sqx[:])
        sqa_sb = consts.tile([P, 2], F32)
        nc.gpsimd.dma_start(out=sqa_sb, in_=sqa[:])
        sk_sb = consts.tile([P, 1], F32)
        nc.gpsimd.dma_start(out=sk_sb, in_=sk[:])
        cos_sb = consts.tile([P, n], BF16)
        nc.gpsimd.dma_start(out=cos_sb, in_=cosT[:])
        sin_sb = consts.tile([P, n], BF16)
        nc.gpsimd.dma_start(out=sin_sb, in_=sinT[:])

        w_sb = {}
        for name, hdl, m in (("wqx", wqx, HPC * DH), ("wqa", wqa, HPC * DH),
                             ("wkvx", wkvx, P), ("wkva", wkva, P)):
            t = consts.tile([P, KT, m], BF16, name=f"w_{name}")
            nc.gpsimd.dma_start(out=t, in_=hdl[:].rearrange("(kt p) m -> p kt m", p=P))
            w_sb[name] = t

        # ---------------- projections ----------------
        # Per modality: Q1 (heads 0-1), Q2 (heads 2-3), KV; chunk-major so the
        # PSUM working set stays at 3 tags x 2 bufs = 6 banks.
        QT = {}   # (mod, half) -> [P, n] bf16, rows [hEven dims | hOdd dims]
        KVX = sb.tile([P, n], BF16, tag="kvx")
        KVA = sb.tile([P, n], BF16, tag="kva")
        ktiles = {}
        for mod, src in (("x", xT), ("a", aT)):
            for ki in range(KT):
                t = sb.tile([P, n], BF16, tag="ktile", bufs=10, name=f"kt_{mod}{ki}")
                nc.sync.dma_start(out=t, in_=src[ki * P:(ki + 1) * P, :])
                ktiles[(mod, ki)] = t

        with tc.tile_pool(name="pj", bufs=1, space="PSUM") as pj:
            for mod, wq_name, wkv_name, kvdst in (
                ("x", "wqx", "wkvx", KVX), ("a", "wqa", "wkva", KVA),
            ):
                q1 = sb.tile([P, n], BF16, tag=f"q1{mod}")
                q2t = sb.tile([P, n], BF16, tag=f"q2{mod}")
                QT[(mod, 0)] = q1
                QT[(mod, 1)] = q2t
                wq_t = w_sb[wq_name]
                wkv_t = w_sb[wkv_name]
                for c in range(NCH):
                    cs = slice(c * CH, (c + 1) * CH)
                    ps1 = pj.tile([P, CH], F32, tag="p1", bufs=2)
                    ps2 = pj.tile([P, CH], F32, tag="p2", bufs=2)
                    psk = pj.tile([P, CH], F32, tag="pk", bufs=2)
                    for ki in range(KT):
                        mv = ktiles[(mod, ki)][:, cs]
                        st = (ki == 0)
                        sp = (ki == KT - 1)
                        nc.tensor.matmul(ps1, wq_t[:, ki, 0:P], mv, start=st, stop=sp)
                        nc.tensor.matmul(ps2, wq_t[:, ki, P:2 * P], mv, start=st, stop=sp)
                        nc.tensor.matmul(psk, wkv_t[:, ki, :], mv, start=st, stop=sp)
                    nc.vector.tensor_copy(q1[:, cs], ps1)
                    nc.vector.tensor_copy(q2t[:, cs], ps2)
                    nc.vector.tensor_copy(kvdst[:, cs], psk)

        # ---------------- V transpose ----------------
        # V_jt [j, d]: cols 0:64 = v_x (KVX rows 64:128), cols 64:128 = v_a
        # (KVA rows 0:64)
        V = []
        with tc.tile_pool(name="vt", bufs=2, space="PSUM") as vtp:
            for jt in range(NJT):
                js = slice(jt * P, (jt + 1) * P)
                psv1 = vtp.tile([P, DH], BF16, tag="v1")
                psv2 = vtp.tile([P, DH], BF16, tag="v2")
                nc.tensor.transpose(psv1, KVX[DH:P, js], ident[DH:P, DH:P])
                nc.tensor.transpose(psv2, KVA[0:DH, js], ident[0:DH, 0:DH])
                vj = sb.tile([P, P], BF16, tag="vsb", bufs=NJT)
                nc.vector.tensor_copy(vj[:, 0:DH], psv1)
                nc.vector.tensor_copy(vj[:, DH:P], psv2)
                V.append(vj)

        # ---------------- qk-norm + per-head rotary layout ----------------
        # Per-head tiles: qh[h] rows [x-half; a-half], qsw[h] rows
        # [a-half; x-half]. The stt writes whichever target matches the source
        # partition range; the companion half is a SBUF->SBUF DMA copy.
        QH = [sb.tile([P, n], BF16, tag=f"qh{h}", name=f"qh{h}") for h in range(HPC)]
        QSW = [sb.tile([P, n], BF16, tag=f"qsw{h}", name=f"qsw{h}") for h in range(HPC)]
        KH = sb.tile([P, n], BF16, tag="kh")
        KSW = sb.tile([P, n], BF16, tag="ksw")

        with tc.tile_pool(name="nm", bufs=1, space="PSUM") as nm:
            def norm_stream(src, dst_list, sc, r0, r1):
                # src rows r0:r1 (one or two 64-row groups) -> sumsq via
                # bdiag matmul -> sqrt (in PSUM) -> reciprocal -> stt into
                # dst slices (each (tile, rows) in dst_list covers r0:r1).
                q2 = sb.tile([P, n], BF16, tag="sq", bufs=2)
                nc.vector.tensor_mul(q2[r0:r1, :], src[r0:r1, :], src[r0:r1, :])
                for c in range(NCH):
                    cs = slice(c * CH, (c + 1) * CH)
                    psr = nm.tile([P, CH], F32, tag="r", bufs=2)
                    prc = nm.tile([P, CH], F32, tag="rc", bufs=2)
                    nc.tensor.matmul(psr[r0:r1, :], ones[r0:r1, 0:r1 - r0],
                                     q2[r0:r1, cs], start=True, stop=True)
                    nc.scalar.activation(psr[r0:r1, :], psr[r0:r1, :], AF.Sqrt,
                                         bias=eps_sb[r0:r1, :], scale=1.0)
                    nc.vector.reciprocal(prc[r0:r1, :], psr[r0:r1, :])
                    for dst, dr0 in dst_list:
                        nc.vector.scalar_tensor_tensor(
                            dst[dr0:dr0 + (r1 - r0), cs], src[r0:r1, cs],
                            sc, prc[r0:r1, :], op0=ALU.mult, op1=ALU.mult)

            # Q streams: head h -> x-half from QT[(x, h//2)] rows (h%2)*64,
            # a-half from QT[(a, h//2)] same rows.
            for h in range(HPC):
                r0 = (h % 2) * DH
                r1 = r0 + DH
                col = h // 2
                # x-half -> qh[h][0:64] (direct if r0==0) else qsw[h][64:128]
                if h % 2 == 0:
                    norm_stream(QT[("x", col)], [(QH[h], 0)],
                                sqx_sb[r0:r1, col:col + 1], r0, r1)
                    nc.sync.dma_start(out=QSW[h][DH:P, :], in_=QH[h][0:DH, :])
                    norm_stream(QT[("a", col)], [(QSW[h], 0)],
                                sqa_sb[r0:r1, col:col + 1], r0, r1)
                    nc.sync.dma_start(out=QH[h][DH:P, :], in_=QSW[h][0:DH, :])
                else:
                    norm_stream(QT[("x", col)], [(QSW[h], DH)],
                                sqx_sb[r0:r1, col:col + 1], r0, r1)
                    nc.sync.dma_start(out=QH[h][0:DH, :], in_=QSW[h][DH:P, :])
                    norm_stream(QT[("a", col)], [(QH[h], DH)],
                                sqa_sb[r0:r1, col:col + 1], r0, r1)
                    nc.sync.dma_start(out=QSW[h][0:DH, :], in_=QH[h][DH:P, :])
            # K: kx = KVX rows 0:64 -> kh[0:64]; ka = KVA rows 64:128 ->
            # kh[64:128]; swapped companions by DMA.
            norm_stream(KVX, [(KH, 0)], sk_sb[0:DH, :], 0, DH)
            nc.sync.dma_start(out=KSW[DH:P, :], in_=KH[0:DH, :])
            norm_stream(KVA, [(KH, DH)], sk_sb[DH:P, :], DH, P)
            nc.sync.dma_start(out=KSW[0:DH, :], in_=KH[DH:P, :])

        # ---------------- rotary ----------------
        # rot(t) = t*cos + t_halfswapped*sin_signed
        # sin_sb carries the sign: rows 0:64 = -sin64, rows 64:128 = +sin64.
        qrot = []
        for h in range(HPC):
            tcos = sb.tile([P, n], BF16, tag="tcos", bufs=2)
            tsin = sb.tile([P, n], BF16, tag="tsin", bufs=2)
            nc.vector.tensor_mul(tcos, QH[h], cos_sb)
            nc.vector.tensor_mul(tsin, QSW[h], sin_sb)
            nc.vector.tensor_add(QH[h], tcos, tsin)
            qrot.append(QH[h])
        tcos = sb.tile([P, n], BF16, tag="tcos", bufs=2)
        tsin = sb.tile([P, n], BF16, tag="tsin", bufs=2)
        nc.vector.tensor_mul(tcos, KH, cos_sb)
        nc.vector.tensor_mul(tsin, KSW, sin_sb)
        krot = KH
        nc.vector.tensor_add(krot, tcos, tsin)

        if stage == 1:
            # dump projections + V
            nc.sync.dma_start(out=out[0], in_=QT[("x", 0)])
            nc.sync.dma_start(out=out[1], in_=QT[("a", 0)])
            nc.sync.dma_start(out=out[2], in_=KVX)
            for jt in range(NJT):
                nc.sync.dma_start(out=out[3][:, jt * P:(jt + 1) * P], in_=V[jt])
        elif stage == 2:
            # dump rotary q0/q1, krot, V
            nc.sync.dma_start(out=out[0], in_=qrot[0])
            nc.sync.dma_start(out=out[1], in_=qrot[1])
            nc.sync.dma_start(out=out[2], in_=krot)
            for jt in range(NJT):
                nc.sync.dma_start(out=out[3][:, jt * P:(jt + 1) * P], in_=V[jt])

        # ---------------- attention ----------------
        with tc.tile_pool(name="at", bufs=1, space="PSUM") as at:
          if stage == 0:
            for h in range(HPC):
                qr = qrot[h]
                for su in range(NSU):
                    ps_o = at.tile([P, SU], F32, tag="o", bufs=1)
                    ps_e = at.tile([P, SU], F32, tag="e", bufs=1)
                    for jt in range(NJT):
                        js = slice(jt * P, (jt + 1) * P)
                        ps_s = at.tile([P, SU], F32, tag="s", bufs=2)
                        for cc in range(SUC):
                            el = slice(cc * CH, (cc + 1) * CH)
                            il = slice(su * SU + cc * CH, su * SU + (cc + 1) * CH)
                            nc.tensor.matmul(ps_s[:, el], krot[:, js], qr[:, il],
                                             start=True, stop=True)
                        es = sb.tile([P, SU], BF16, tag="es", bufs=3)
                        nc.scalar.activation(es, ps_s, AF.Exp, bias=0.0,
                                             scale=SM_SCALE)
                        for cc in range(SUC):
                            el = slice(cc * CH, (cc + 1) * CH)
                            nc.tensor.matmul(ps_e[:, el], ones, es[:, el],
                                             start=(jt == 0), stop=(jt == NJT - 1))
                        for cc in range(SUC):
                            el = slice(cc * CH, (cc + 1) * CH)
                            nc.tensor.matmul(ps_o[:, el], V[jt], es[:, el],
                                             start=(jt == 0), stop=(jt == NJT - 1))
                    rec = sb.tile([P, SU], F32, tag="rec", bufs=2)
                    nc.vector.reciprocal_approx_fast(out=rec, in_=ps_e)
                    on = sb.tile([P, SU], BF16, tag="on", bufs=2)
                    nc.vector.tensor_mul(on, ps_o, rec)
                    nc.sync.dma_start(out=out[h, :, su * SU:(su + 1) * SU], in_=on)
    nc.finalize()
    return nc


# ---------------------------------------------------------------------------
# host side
# ---------------------------------------------------------------------------

_NC_CACHE = {}


def get_nc(n=N, nb=B):
    key = n
    if key not in _NC_CACHE:
        _NC_CACHE[key] = build_nc(n)
    return _NC_CACHE[key]


def rotary_tables(n):
    inv_freq = 1.0 / (10000.0 ** (np.arange(0, ROT, 2, dtype=np.float64) / ROT))
    freqs = np.outer(np.arange(n, dtype=np.float64), inv_freq)  # [n, 64]
    cos64 = np.cos(freqs).T.astype(np.float32)                  # [64, n]
    sin64 = np.sin(freqs).T.astype(np.float32)
    cosT = np.ascontiguousarray(np.concatenate([cos64, cos64], 0)).astype(NPBF)
    sinT = np.ascontiguousarray(np.concatenate([-sin64, sin64], 0)).astype(NPBF)
    return cosT, sinT


def prep_in_maps(inputs, n=N, nb=B, ncores=NCORES):
    g = {k: np.asarray(v, dtype=np.float32) for k, v in inputs.items()}
    xT = [np.ascontiguousarray(g["x"][b].T).astype(NPBF) for b in range(nb)]
    aT = [np.ascontiguousarray(g["a"][b].T).astype(NPBF) for b in range(nb)]
    wkvx = np.ascontiguousarray(g["Wkv_x"].T).astype(NPBF)          # cols [kx|vx]
    wkva = np.ascontiguousarray(
        np.concatenate([g["Wkv_a"][DH:2 * DH], g["Wkv_a"][0:DH]], 0).T
    ).astype(NPBF)                                                  # cols [va|ka]
    sk = np.ascontiguousarray(
        np.concatenate([g["kx_scale"][0, 0], g["ka_scale"][0, 0]])[:, None]
    ).astype(np.float32)
    cosT, sinT = rotary_tables(n)

    in_maps = []
    for c in range(ncores):
        b = c // (ncores // nb)
        h0 = (c % (ncores // nb)) * HPC
        m = dict(xT=xT[b], aT=aT[b], wkvx=wkvx, wkva=wkva, sk=sk,
                 cosT=cosT, sinT=sinT)
        m["wqx"] = np.ascontiguousarray(
            g["Wq_x"][h0 * DH:(h0 + HPC) * DH].T).astype(NPBF)
        m["wqa"] = np.ascontiguousarray(
            g["Wq_a"][h0 * DH:(h0 + HPC) * DH].T).astype(NPBF)
        m["sqx"] = np.ascontiguousarray(np.stack(
            [np.concatenate([g["qx_scale"][h0 + 2 * t, 0],
                             g["qx_scale"][h0 + 2 * t + 1, 0]]) for t in range(2)],
            axis=1)).astype(np.float32)
        m["sqa"] = np.ascontiguousarray(np.stack(
            [np.concatenate([g["qa_scale"][h0 + 2 * t, 0],
                             g["qa_scale"][h0 + 2 * t + 1, 0]]) for t in range(2)],
            axis=1)).astype(np.float32)
        in_maps.append(m)
    return in_maps


def gather_out(results, n=N, nb=B, ncores=NCORES):
    full = np.empty((nb, n, HEADS * ROT), np.float32)
    for c in range(ncores):
        b = c // (ncores // nb)
        h0 = (c % (ncores // nb)) * HPC
        o = np.asarray(results[c]["out"]).astype(np.float32)  # [HPC, ROT, n]
        for h in range(HPC):
            gh = h0 + h
            full[b, :, gh * ROT:(gh + 1) * ROT] = o[h].T
    return full


def kernel(**inputs):
    from concourse.bass_utils import run_bass_kernel_spmd
    nc = get_nc(N, B)
    in_maps = prep_in_maps(inputs, N, B, NCORES)
    res = run_bass_kernel_spmd(nc, in_maps, list(range(NCORES)))
    return gather_out(res.results, N, B, NCORES)


if __name__ == "__main__":
    build_nc(256)
    print("build ok")


# revision 12
# speedup vs baseline: 2.8882x; 1.2446x over previous
"""CMAttention Trainium2 kernel (8-core SPMD, bf16 compute).

Reference computation (per nn_CMAttention):
  q_x = (x @ Wq_x.T)  -> [b, 16, n, 64],  q_a likewise
  kv_x = x @ Wkv_x.T -> k_x, v_x [b, 1, n, 64] (single shared KV head), kv_a likewise
  l2norm + learned scales on q_x/q_a (per head) and k_x/k_a (shared)
  q = concat(q_x, q_a) [b,16,n,128]; k, v likewise [b,1,n,128]
  rotary(q, k) over the 128-dim concat axis; SDPA with softmax over keys.

Sharding: each core owns ONE batch (core//4) and FOUR heads ((core%4)*4 ..).
The shared KV projection is computed replicated on the 4 cores of a batch.

Device-side layout: everything is computed "transposed" (feature dim on
partitions, sequence on the free axis). All matmuls run in bf16 (fp32 matmul
is 4 cycles/column on TRN2; bf16 is 1), accumulating in fp32 PSUM. Softmax
runs on S^T tiles: the row-sum over keys is a ones-matmul partition reduction
accumulated in PSUM; no max subtraction is needed because q/k rows are
l2-normalized (|scores*scale| <= ~0.2 for unit scales).

Per-head rotary layout trick: the qk-norm scalar_tensor_tensor writes its
output DIRECTLY into the per-head [x-half; a-half] rotary tiles wherever the
partition ranges line up (DVE ops need matching start partitions); the
mismatched half of each tile is filled with one SBUF->SBUF DMA from its
companion tile.
"""

import numpy as np
import ml_dtypes
from contextlib import ExitStack

import concourse.bass as bass
from concourse import bacc
import concourse.mybir as mybir
import concourse.tile as tile
from concourse.masks import make_identity

F32 = mybir.dt.float32
BF16 = mybir.dt.bfloat16
AF = mybir.ActivationFunctionType
ALU = mybir.AluOpType
NPBF = ml_dtypes.bfloat16

P = 128
B, N, DIM = 2, 2048, 1024
HEADS, DH, ROT = 16, 64, 128
NCORES = 8
HPC = 4                     # heads per core (one batch per core)
KT = DIM // P               # 8 contraction tiles
SM_SCALE = float(1.0 / np.sqrt(ROT))


def build_nc(n=N, stage=0):
    CH = min(512, n)        # fp32 PSUM bank = 512 floats
    NCH = n // CH
    SU = min(1024, n)       # attention superunit width (2 PSUM banks)
    NSU = n // SU
    SUC = SU // CH
    NJT = n // P            # key tiles

    nc = bacc.Bacc()
    dp = nc.declare_dram_parameter
    xT = dp("xT", [DIM, n], BF16, isOutput=False)
    aT = dp("aT", [DIM, n], BF16, isOutput=False)
    wqx = dp("wqx", [DIM, HPC * DH], BF16, isOutput=False)
    wqa = dp("wqa", [DIM, HPC * DH], BF16, isOutput=False)
    wkvx = dp("wkvx", [DIM, P], BF16, isOutput=False)  # cols [k_x | v_x]
    wkva = dp("wkva", [DIM, P], BF16, isOutput=False)  # cols [v_a | k_a] (host-permuted)
    sqx = dp("sqx", [P, 2], F32, isOutput=False)       # col t: heads (2t, 2t+1)
    sqa = dp("sqa", [P, 2], F32, isOutput=False)
    sk = dp("sk", [P, 1], F32, isOutput=False)         # rows 0:64 kx_scale, 64:128 ka_scale
    cosT = dp("cosT", [P, n], BF16, isOutput=False)    # [cos64; cos64]
    sinT = dp("sinT", [P, n], BF16, isOutput=False)    # [-sin64; sin64]
    out = dp("out", [HPC, ROT, n], BF16, isOutput=True)

    with ExitStack() as ctx:
        tc = ctx.enter_context(tile.TileContext(nc))
        consts = ctx.enter_context(tc.tile_pool(name="consts", bufs=1))
        sb = ctx.enter_context(tc.tile_pool(name="sb", bufs=1))

        ones = consts.tile([P, P], BF16)
        nc.vector.memset(ones, 1.0)
        # block-diagonal ones: per-64-group partition sums in one matmul
        bdiag = consts.tile([P, P], BF16)
        nc.vector.memset(bdiag, 0.0)
        nc.vector.memset(bdiag[0:DH, 0:DH], 1.0)
        nc.vector.memset(bdiag[DH:P, DH:P], 1.0)
        eps_sb = consts.tile([P, 1], F32)
        nc.vector.memset(eps_sb, 1e-24)
        ident = consts.tile([P, P], BF16)
        make_identity(nc, ident)

        sqx_sb = consts.tile([P, 2], F32)
        nc.gpsimd.dma_start(out=sqx_sb, in_=sqx[:])
        sqa_sb = consts.tile([P, 2], F32)
        nc.gpsimd.dma_start(out=sqa_sb, in_=sqa[:])
        sk_sb = consts.tile([P, 1], F32)
        nc.gpsimd.dma_start(out=sk_sb, in_=sk[:])
        cos_sb = consts.tile([P, n], BF16)
        nc.sync.dma_start(out=cos_sb, in_=cosT[:])
        sin_sb = consts.tile([P, n], BF16)
        nc.sync.dma_start(out=sin_sb, in_=sinT[:])

        w_sb = {}
        for name, hdl, m in (("wqx", wqx, HPC * DH), ("wqa", wqa, HPC * DH),
                             ("wkvx", wkvx, P), ("wkva", wkva, P)):
            t = consts.tile([P, KT, m], BF16, name=f"w_{name}")
            nc.sync.dma_start(out=t, in_=hdl[:].rearrange("(kt p) m -> p kt m", p=P))
            w_sb[name] = t

        # ---------------- projections ----------------
        # Per modality: Q1 (heads 0-1), Q2 (heads 2-3), KV; chunk-major so the
        # PSUM working set stays at 3 tags x 2 bufs = 6 banks.
        QT = {}   # (mod, half) -> [P, n] bf16, rows [hEven dims | hOdd dims]
        KVX = sb.tile([P, n], BF16, tag="kvx")
        KVA = sb.tile([P, n], BF16, tag="kva")
        ktiles = {}
        for mod, src in (("x", xT), ("a", aT)):
            for ki in range(KT):
                t = sb.tile([P, n], BF16, tag="ktile", bufs=10, name=f"kt_{mod}{ki}")
                nc.sync.dma_start(out=t, in_=src[ki * P:(ki + 1) * P, :])
                ktiles[(mod, ki)] = t

        with tc.tile_pool(name="pj", bufs=1, space="PSUM") as pj:
            for mod, wq_name, wkv_name, kvdst in (
                ("x", "wqx", "wkvx", KVX), ("a", "wqa", "wkva", KVA),
            ):
                q1 = sb.tile([P, n], BF16, tag=f"q1{mod}")
                q2t = sb.tile([P, n], BF16, tag=f"q2{mod}")
                QT[(mod, 0)] = q1
                QT[(mod, 1)] = q2t
                wq_t = w_sb[wq_name]
                wkv_t = w_sb[wkv_name]
                for c in range(NCH):
                    cs = slice(c * CH, (c + 1) * CH)
                    ps1 = pj.tile([P, CH], F32, tag="p1", bufs=2)
                    ps2 = pj.tile([P, CH], F32, tag="p2", bufs=2)
                    psk = pj.tile([P, CH], F32, tag="pk", bufs=2)
                    for ki in range(KT):
                        mv = ktiles[(mod, ki)][:, cs]
                        st = (ki == 0)
                        sp = (ki == KT - 1)
                        nc.tensor.matmul(ps1, wq_t[:, ki, 0:P], mv, start=st, stop=sp)
                        nc.tensor.matmul(ps2, wq_t[:, ki, P:2 * P], mv, start=st, stop=sp)
                        nc.tensor.matmul(psk, wkv_t[:, ki, :], mv, start=st, stop=sp)
                    nc.vector.tensor_copy(q1[:, cs], ps1)
                    nc.vector.tensor_copy(q2t[:, cs], ps2)
                    nc.vector.tensor_copy(kvdst[:, cs], psk)

        # ---------------- V transpose ----------------
        # V_jt [j, d]: cols 0:64 = v_x (KVX rows 64:128), cols 64:128 = v_a
        # (KVA rows 0:64)
        V = []
        with tc.tile_pool(name="vt", bufs=2, space="PSUM") as vtp:
            for jt in range(NJT):
                js = slice(jt * P, (jt + 1) * P)
                psv1 = vtp.tile([P, DH], BF16, tag="v1")
                psv2 = vtp.tile([P, DH], BF16, tag="v2")
                nc.tensor.transpose(psv1, KVX[DH:P, js], ident[DH:P, DH:P])
                nc.tensor.transpose(psv2, KVA[0:DH, js], ident[0:DH, 0:DH])
                vj = sb.tile([P, P], BF16, tag="vsb", bufs=NJT)
                nc.vector.tensor_copy(vj[:, 0:DH], psv1)
                nc.vector.tensor_copy(vj[:, DH:P], psv2)
                V.append(vj)

        # ---------------- qk-norm + per-head rotary layout ----------------
        # Per-head tiles: qh[h] rows [x-half; a-half], qsw[h] rows
        # [a-half; x-half]. The stt writes whichever target matches the source
        # partition range; the companion half is a SBUF->SBUF DMA copy.
        #
        # All streams are emitted batched by op type (squares -> sum matmuls
        # -> rsqrt -> stt -> swap DMAs) so the per-stream PE->ACT->DVE chains
        # pipeline across streams instead of serializing.
        QH = [sb.tile([P, n], BF16, tag=f"qh{h}", name=f"qh{h}") for h in range(HPC)]
        QSW = [sb.tile([P, n], BF16, tag=f"qsw{h}", name=f"qsw{h}") for h in range(HPC)]
        KH = sb.tile([P, n], BF16, tag="kh")
        KSW = sb.tile([P, n], BF16, tag="ksw")

        # streams: (src_tile, sq_tile, rows r0, scale_ap, direct target
        # (tile, dr0), companion DMA (dst_tile, dst_r0, src_r0-of-direct))
        sq_tiles = {}
        for key, src in (("qx0", QT[("x", 0)]), ("qx1", QT[("x", 1)]),
                         ("qa0", QT[("a", 0)]), ("qa1", QT[("a", 1)])):
            q2 = sb.tile([P, n], BF16, tag=f"sq_{key}")
            nc.vector.tensor_mul(q2, src, src)
            sq_tiles[key] = q2
        k2 = sb.tile([P, n], BF16, tag="sq_k")
        nc.vector.tensor_mul(k2[0:DH, :], KVX[0:DH, :], KVX[0:DH, :])
        nc.vector.tensor_mul(k2[DH:P, :], KVA[DH:P, :], KVA[DH:P, :])
        sq_tiles["k"] = k2

        streams = []
        for h in range(HPC):
            r0 = (h % 2) * DH
            col = h // 2
            if h % 2 == 0:
                streams.append((QT[("x", col)], sq_tiles[f"qx{col}"], r0,
                                sqx_sb[r0:r0 + DH, col:col + 1], (QH[h], 0)))
                streams.append((QT[("a", col)], sq_tiles[f"qa{col}"], r0,
                                sqa_sb[r0:r0 + DH, col:col + 1], (QSW[h], 0)))
            else:
                streams.append((QT[("x", col)], sq_tiles[f"qx{col}"], r0,
                                sqx_sb[r0:r0 + DH, col:col + 1], (QSW[h], DH)))
                streams.append((QT[("a", col)], sq_tiles[f"qa{col}"], r0,
                                sqa_sb[r0:r0 + DH, col:col + 1], (QH[h], DH)))
        streams.append((KVX, k2, 0, sk_sb[0:DH, :], (KH, 0)))
        streams.append((KVA, k2, DH, sk_sb[DH:P, :], (KH, DH)))

        with tc.tile_pool(name="nm", bufs=1, space="PSUM") as nm:
            # per (stream, chunk): mm -> rsqrt -> stt, interleaved so the
            # PE/ACT/DVE pipelines stay full across chunks/streams
            for si, (src, q2, r0, sc, (dst, dr0)) in enumerate(streams):
                r1 = r0 + DH
                for c in range(NCH):
                    cs = slice(c * CH, (c + 1) * CH)
                    psr = nm.tile([P, CH], F32, tag="r", bufs=4)
                    prc = sb.tile([P, CH], F32, tag="prc", bufs=4)
                    nc.tensor.matmul(psr[r0:r1, :], ones[r0:r1, 0:DH],
                                     q2[r0:r1, cs], start=True, stop=True)
                    if r0 == 0:
                        # sqrt in PSUM, then fast approx reciprocal (only
                        # correct at base partition 0 on HW)
                        nc.scalar.activation(psr[r0:r1, :], psr[r0:r1, :],
                                             AF.Sqrt, bias=eps_sb[r0:r1, :],
                                             scale=1.0)
                        nc.vector.reciprocal_approx_fast(out=prc[r0:r1, :],
                                                         in_=psr[r0:r1, :])
                    else:
                        # rsqrt = exp(-0.5 * ln(ss)) on the scalar engine
                        nc.scalar.activation(psr[r0:r1, :], psr[r0:r1, :],
                                             AF.Ln, bias=eps_sb[r0:r1, :],
                                             scale=1.0)
                        nc.scalar.activation(prc[r0:r1, :], psr[r0:r1, :],
                                             AF.Exp, bias=0.0, scale=-0.5)
                    nc.vector.scalar_tensor_tensor(
                        dst[dr0:dr0 + DH, cs], src[r0:r1, cs],
                        sc, prc[r0:r1, :], op0=ALU.mult, op1=ALU.mult)

        # phase 3: companion-half swap DMAs
        for h in range(HPC):
            if h % 2 == 0:
                nc.sync.dma_start(out=QSW[h][DH:P, :], in_=QH[h][0:DH, :])
                nc.sync.dma_start(out=QH[h][DH:P, :], in_=QSW[h][0:DH, :])
            else:
                nc.sync.dma_start(out=QH[h][0:DH, :], in_=QSW[h][DH:P, :])
                nc.sync.dma_start(out=QSW[h][0:DH, :], in_=QH[h][DH:P, :])
        nc.sync.dma_start(out=KSW[DH:P, :], in_=KH[0:DH, :])
        nc.sync.dma_start(out=KSW[0:DH, :], in_=KH[DH:P, :])

        # ---------------- rotary ----------------
        # rot(t) = t*cos + t_halfswapped*sin_signed
        # sin_sb carries the sign: rows 0:64 = -sin64, rows 64:128 = +sin64.
        qrot = []
        for h in range(HPC):
            tcos = sb.tile([P, n], BF16, tag="tcos", bufs=2)
            tsin = sb.tile([P, n], BF16, tag="tsin", bufs=2)
            nc.vector.tensor_mul(tcos, QH[h], cos_sb)
            nc.vector.tensor_mul(tsin, QSW[h], sin_sb)
            nc.vector.tensor_add(QH[h], tcos, tsin)
            qrot.append(QH[h])
        tcos = sb.tile([P, n], BF16, tag="tcos", bufs=2)
        tsin = sb.tile([P, n], BF16, tag="tsin", bufs=2)
        nc.vector.tensor_mul(tcos, KH, cos_sb)
        nc.vector.tensor_mul(tsin, KSW, sin_sb)
        krot = KH
        nc.vector.tensor_add(krot, tcos, tsin)

        if stage == 1:
            # dump projections + V
            nc.sync.dma_start(out=out[0], in_=QT[("x", 0)])
            nc.sync.dma_start(out=out[1], in_=QT[("a", 0)])
            nc.sync.dma_start(out=out[2], in_=KVX)
            for jt in range(NJT):
                nc.sync.dma_start(out=out[3][:, jt * P:(jt + 1) * P], in_=V[jt])
        elif stage == 2:
            # dump rotary q0/q1, krot, V
            nc.sync.dma_start(out=out[0], in_=qrot[0])
            nc.sync.dma_start(out=out[1], in_=qrot[1])
            nc.sync.dma_start(out=out[2], in_=krot)
            for jt in range(NJT):
                nc.sync.dma_start(out=out[3][:, jt * P:(jt + 1) * P], in_=V[jt])

        # ---------------- attention ----------------
        with tc.tile_pool(name="at", bufs=1, space="PSUM") as at:
          if stage == 0:
            def emit_scores(h, su, jt):
                js = slice(jt * P, (jt + 1) * P)
                ps_s = at.tile([P, SU], F32, tag="s", bufs=2, name=f"s{h}_{su}_{jt}")
                for cc in range(SUC):
                    el = slice(cc * CH, (cc + 1) * CH)
                    il = slice(su * SU + cc * CH, su * SU + (cc + 1) * CH)
                    nc.tensor.matmul(ps_s[:, el], krot[:, js], qrot[h][:, il],
                                     start=True, stop=True)
                return ps_s

            # software pipeline: scores(jt+1) is emitted (PE queue) before the
            # exp-dependent ones/AV matmuls of jt, so the PE never waits on
            # the scalar engine's exp round-trip.
            hsu = [(h, su) for h in range(HPC) for su in range(NSU)]
            for h, su in hsu:
                ps_o = at.tile([P, SU], F32, tag="o", bufs=1, name=f"o{h}_{su}")
                ps_e = at.tile([P, SU], F32, tag="e", bufs=1, name=f"e{h}_{su}")
                ps_s = emit_scores(h, su, 0)
                for jt in range(NJT):
                    es = sb.tile([P, SU], BF16, tag="es", bufs=3)
                    nc.scalar.activation(es, ps_s, AF.Exp, bias=0.0,
                                         scale=SM_SCALE)
                    if jt + 1 < NJT:
                        ps_s = emit_scores(h, su, jt + 1)
                    for cc in range(SUC):
                        el = slice(cc * CH, (cc + 1) * CH)
                        nc.tensor.matmul(ps_e[:, el], ones, es[:, el],
                                         start=(jt == 0), stop=(jt == NJT - 1))
                    for cc in range(SUC):
                        el = slice(cc * CH, (cc + 1) * CH)
                        nc.tensor.matmul(ps_o[:, el], V[jt], es[:, el],
                                         start=(jt == 0), stop=(jt == NJT - 1))
                rec = sb.tile([P, SU], F32, tag="rec", bufs=2)
                nc.vector.reciprocal_approx_fast(out=rec, in_=ps_e)
                on = sb.tile([P, SU], BF16, tag="on", bufs=2)
                nc.vector.tensor_mul(on, ps_o, rec)
                nc.sync.dma_start(out=out[h, :, su * SU:(su + 1) * SU], in_=on)
    nc.finalize()
    return nc


# ---------------------------------------------------------------------------
# host side
# ---------------------------------------------------------------------------

_NC_CACHE = {}


def get_nc(n=N, nb=B):
    key = n
    if key not in _NC_CACHE:
        _NC_CACHE[key] = build_nc(n)
    return _NC_CACHE[key]


def rotary_tables(n):
    inv_freq = 1.0 / (10000.0 ** (np.arange(0, ROT, 2, dtype=np.float64) / ROT))
    freqs = np.outer(np.arange(n, dtype=np.float64), inv_freq)  # [n, 64]
    cos64 = np.cos(freqs).T.astype(np.float32)                  # [64, n]
    sin64 = np.sin(freqs).T.astype(np.float32)
    cosT = np.ascontiguousarray(np.concatenate([cos64, cos64], 0)).astype(NPBF)
    sinT = np.ascontiguousarray(np.concatenate([-sin64, sin64], 0)).astype(NPBF)
    return cosT, sinT


def prep_in_maps(inputs, n=N, nb=B, ncores=NCORES):
    g = {k: np.asarray(v, dtype=np.float32) for k, v in inputs.items()}
    xT = [np.ascontiguousarray(g["x"][b].T).astype(NPBF) for b in range(nb)]
    aT = [np.ascontiguousarray(g["a"][b].T).astype(NPBF) for b in range(nb)]
    wkvx = np.ascontiguousarray(g["Wkv_x"].T).astype(NPBF)          # cols [kx|vx]
    wkva = np.ascontiguousarray(
        np.concatenate([g["Wkv_a"][DH:2 * DH], g["Wkv_a"][0:DH]], 0).T
    ).astype(NPBF)                                                  # cols [va|ka]
    sk = np.ascontiguousarray(
        np.concatenate([g["kx_scale"][0, 0], g["ka_scale"][0, 0]])[:, None]
    ).astype(np.float32)
    cosT, sinT = rotary_tables(n)

    in_maps = []
    for c in range(ncores):
        b = c // (ncores // nb)
        h0 = (c % (ncores // nb)) * HPC
        m = dict(xT=xT[b], aT=aT[b], wkvx=wkvx, wkva=wkva, sk=sk,
                 cosT=cosT, sinT=sinT)
        m["wqx"] = np.ascontiguousarray(
            g["Wq_x"][h0 * DH:(h0 + HPC) * DH].T).astype(NPBF)
        m["wqa"] = np.ascontiguousarray(
            g["Wq_a"][h0 * DH:(h0 + HPC) * DH].T).astype(NPBF)
        m["sqx"] = np.ascontiguousarray(np.stack(
            [np.concatenate([g["qx_scale"][h0 + 2 * t, 0],
                             g["qx_scale"][h0 + 2 * t + 1, 0]]) for t in range(2)],
            axis=1)).astype(np.float32)
        m["sqa"] = np.ascontiguousarray(np.stack(
            [np.concatenate([g["qa_scale"][h0 + 2 * t, 0],
                             g["qa_scale"][h0 + 2 * t + 1, 0]]) for t in range(2)],
            axis=1)).astype(np.float32)
        in_maps.append(m)
    return in_maps


def gather_out(results, n=N, nb=B, ncores=NCORES):
    full = np.empty((nb, n, HEADS * ROT), np.float32)
    for c in range(ncores):
        b = c // (ncores // nb)
        h0 = (c % (ncores // nb)) * HPC
        o = np.asarray(results[c]["out"]).astype(np.float32)  # [HPC, ROT, n]
        for h in range(HPC):
            gh = h0 + h
            full[b, :, gh * ROT:(gh + 1) * ROT] = o[h].T
    return full


def kernel(**inputs):
    from concourse.bass_utils import run_bass_kernel_spmd
    nc = get_nc(N, B)
    in_maps = prep_in_maps(inputs, N, B, NCORES)
    res = run_bass_kernel_spmd(nc, in_maps, list(range(NCORES)))
    return gather_out(res.results, N, B, NCORES)


if __name__ == "__main__":
    build_nc(256)
    print("build ok")


# revision 25
# speedup vs baseline: 3.6440x; 1.2617x over previous
"""CMAttention Trainium2 kernel (8-core SPMD, bf16 compute).

Reference computation (per nn_CMAttention):
  q_x = (x @ Wq_x.T)  -> [b, 16, n, 64],  q_a likewise
  kv_x = x @ Wkv_x.T -> k_x, v_x [b, 1, n, 64] (single shared KV head), kv_a likewise
  l2norm + learned scales on q_x/q_a (per head) and k_x/k_a (shared)
  q = concat(q_x, q_a) [b,16,n,128]; k, v likewise [b,1,n,128]
  rotary(q, k) over the 128-dim concat axis; SDPA with softmax over keys.

Sharding: each core owns ONE batch (core//4) and FOUR heads ((core%4)*4 ..).
The shared KV projection is computed replicated on the 4 cores of a batch.

Device-side layout: everything is computed "transposed" (feature dim on
partitions, sequence on the free axis). All matmuls run in bf16 (fp32 matmul
is 4 cycles/column on TRN2; bf16 is 1), accumulating in fp32 PSUM. Softmax
runs on S^T tiles: the row-sum over keys is a ones-matmul partition reduction
accumulated in PSUM; no max subtraction is needed because q/k rows are
l2-normalized (|scores*scale| <= ~0.2 for unit scales).

Per-head rotary layout trick: the qk-norm scalar_tensor_tensor writes its
output DIRECTLY into the per-head [x-half; a-half] rotary tiles wherever the
partition ranges line up (DVE ops need matching start partitions); the
mismatched half of each tile is filled with one SBUF->SBUF DMA from its
companion tile.
"""

import numpy as np
import ml_dtypes
from contextlib import ExitStack

import concourse.bass as bass
from concourse import bacc
import concourse.mybir as mybir
import concourse.tile as tile
from concourse.masks import make_identity

F32 = mybir.dt.float32
BF16 = mybir.dt.bfloat16
AF = mybir.ActivationFunctionType
ALU = mybir.AluOpType
NPBF = ml_dtypes.bfloat16

P = 128
B, N, DIM = 2, 2048, 1024
HEADS, DH, ROT = 16, 64, 128
NCORES = 8
HPC = 4                     # heads per core (one batch per core)
KT = DIM // P               # 8 contraction tiles
SM_SCALE = float(1.0 / np.sqrt(ROT))


def build_nc(n=N, stage=0):
    CH = min(512, n)        # fp32 PSUM bank = 512 floats
    NCH = n // CH
    SU = min(1024, n)       # attention superunit width (2 PSUM banks)
    NSU = n // SU
    SUC = SU // CH
    NJT = n // P            # key tiles

    nc = bacc.Bacc()
    dp = nc.declare_dram_parameter
    xT = dp("xT", [DIM, n], BF16, isOutput=False)
    aT = dp("aT", [DIM, n], BF16, isOutput=False)
    wqx = dp("wqx", [DIM, HPC * DH], BF16, isOutput=False)
    wqa = dp("wqa", [DIM, HPC * DH], BF16, isOutput=False)
    wkvx = dp("wkvx", [DIM, P], BF16, isOutput=False)  # cols [k_x | v_x]
    wkva = dp("wkva", [DIM, P], BF16, isOutput=False)  # cols [k_a | v_a]
    sqx = dp("sqx", [P, 2], F32, isOutput=False)       # col t: heads (2t, 2t+1)
    sqa = dp("sqa", [P, 2], F32, isOutput=False)
    sk = dp("sk", [P, 2], F32, isOutput=False)         # rows 0:64: col0 kx, col1 ka
    cosT = dp("cosT", [P, n], BF16, isOutput=False)    # [cos64; cos64]
    sinT = dp("sinT", [P, n], BF16, isOutput=False)    # [-sin64; sin64]
    out = dp("out", [HPC, ROT, n], BF16, isOutput=True)

    with ExitStack() as ctx:
        tc = ctx.enter_context(tile.TileContext(nc))
        consts = ctx.enter_context(tc.tile_pool(name="consts", bufs=1))
        sb = ctx.enter_context(tc.tile_pool(name="sb", bufs=1))

        ones = consts.tile([P, P], BF16)
        nc.vector.memset(ones, 1.0)
        eps_sb = consts.tile([P, 1], F32)
        nc.vector.memset(eps_sb, 1e-24)
        ident = consts.tile([P, P], BF16)
        make_identity(nc, ident)

        sqx_sb = consts.tile([P, 2], F32)
        nc.gpsimd.dma_start(out=sqx_sb, in_=sqx[:])
        sqa_sb = consts.tile([P, 2], F32)
        nc.gpsimd.dma_start(out=sqa_sb, in_=sqa[:])
        sk_sb = consts.tile([P, 2], F32)
        nc.gpsimd.dma_start(out=sk_sb, in_=sk[:])
        cos_sb = consts.tile([P, n], BF16)
        nc.sync.dma_start(out=cos_sb, in_=cosT[:])
        sin_sb = consts.tile([P, n], BF16)
        nc.sync.dma_start(out=sin_sb, in_=sinT[:])

        w_sb = {}
        for name, hdl, m in (("wqx", wqx, HPC * DH), ("wqa", wqa, HPC * DH),
                             ("wkvx", wkvx, P), ("wkva", wkva, P)):
            t = consts.tile([P, KT, m], BF16, name=f"w_{name}")
            nc.sync.dma_start(out=t, in_=hdl[:].rearrange("(kt p) m -> p kt m", p=P))
            w_sb[name] = t

        # ---------------- projections ----------------
        # Per modality: Q1 (heads 0-1), Q2 (heads 2-3), KV; chunk-major so the
        # PSUM working set stays at 3 tags x 2 bufs = 6 banks.
        QT = {}   # (mod, half) -> [P, n] bf16, rows [hEven dims | hOdd dims]
        KVX = sb.tile([P, n], BF16, tag="kvx")
        KVA = sb.tile([P, n], BF16, tag="kva")
        # chunk-split input loads (c-major): chunk 0 of every k-tile lands
        # first, spread over the DMA queues, so chunk-major matmuls can start
        # after ~1/NCH of the input DMA instead of all of it
        ktiles = {}
        for mod, src in (("x", xT), ("a", aT)):
            for ki in range(KT):
                ktiles[(mod, ki)] = sb.tile([P, n], BF16, tag="ktile", bufs=10,
                                            name=f"kt_{mod}{ki}")
        for c in range(NCH):
            cs = slice(c * CH, (c + 1) * CH)
            for mod, src in (("x", xT), ("a", aT)):
                for ki in range(KT):
                    nc.sync.dma_start(out=ktiles[(mod, ki)][:, cs],
                                      in_=src[ki * P:(ki + 1) * P, cs])

        with tc.tile_pool(name="pj", bufs=1, space="PSUM") as pj:
            for mod, wq_name, wkv_name, kvdst in (
                ("x", "wqx", "wkvx", KVX), ("a", "wqa", "wkva", KVA),
            ):
                q1 = sb.tile([P, n], BF16, tag=f"q1{mod}")
                q2t = sb.tile([P, n], BF16, tag=f"q2{mod}")
                QT[(mod, 0)] = q1
                QT[(mod, 1)] = q2t
                wq_t = w_sb[wq_name]
                wkv_t = w_sb[wkv_name]
                for c in range(NCH):
                    cs = slice(c * CH, (c + 1) * CH)
                    ps1 = pj.tile([P, CH], F32, tag="p1", bufs=2)
                    ps2 = pj.tile([P, CH], F32, tag="p2", bufs=2)
                    psk = pj.tile([P, CH], F32, tag="pk", bufs=2)
                    for ki in range(KT):
                        mv = ktiles[(mod, ki)][:, cs]
                        st = (ki == 0)
                        sp = (ki == KT - 1)
                        nc.tensor.matmul(ps1, wq_t[:, ki, 0:P], mv, start=st, stop=sp)
                        nc.tensor.matmul(ps2, wq_t[:, ki, P:2 * P], mv, start=st, stop=sp)
                        nc.tensor.matmul(psk, wkv_t[:, ki, :], mv, start=st, stop=sp)
                    nc.vector.tensor_copy(q1[:, cs], ps1)
                    nc.vector.tensor_copy(q2t[:, cs], ps2)
                    nc.vector.tensor_copy(kvdst[:, cs], psk)

        # ---------------- V transpose ----------------
        # V_jt [j, d]: cols 0:64 = v_x (KVX rows 64:128), cols 64:128 = v_a
        # (KVA rows 64:128)
        V = []
        with tc.tile_pool(name="vt", bufs=2, space="PSUM") as vtp:
            for jt in range(NJT):
                js = slice(jt * P, (jt + 1) * P)
                psv1 = vtp.tile([P, DH], BF16, tag="v1")
                psv2 = vtp.tile([P, DH], BF16, tag="v2")
                nc.tensor.transpose(psv1, KVX[DH:P, js], ident[DH:P, DH:P])
                nc.tensor.transpose(psv2, KVA[DH:P, js], ident[DH:P, DH:P])
                vj = sb.tile([P, P], BF16, tag="vsb", bufs=NJT)
                nc.vector.tensor_copy(vj[:, 0:DH], psv1)
                nc.vector.tensor_copy(vj[:, DH:P], psv2)
                V.append(vj)

        # ---------------- qk-norm + per-head rotary layout ----------------
        # Per-head tiles: qh[h] rows [x-half; a-half], qsw[h] rows
        # [a-half; x-half]. The stt writes whichever target matches the source
        # partition range; the companion half is a SBUF->SBUF DMA copy.
        #
        # All streams are emitted batched by op type (squares -> sum matmuls
        # -> rsqrt -> stt -> swap DMAs) so the per-stream PE->ACT->DVE chains
        # pipeline across streams instead of serializing.
        QH = [sb.tile([P, n], BF16, tag=f"qh{h}", name=f"qh{h}") for h in range(HPC)]
        QSW = [sb.tile([P, n], BF16, tag=f"qsw{h}", name=f"qsw{h}") for h in range(HPC)]
        KH = sb.tile([P, n], BF16, tag="kh")
        KSW = sb.tile([P, n], BF16, tag="ksw")

        # units: K first (every head's attention needs krot), then heads in
        # order. Each stream: (src, r0, scale, (direct_dst, dst_r0)).
        units = [("k", KH, KSW,
                  [(KVX, 0, sk_sb[0:DH, 0:1], (KH, 0)),
                   (KVA, 0, sk_sb[0:DH, 1:2], (KSW, 0))])]
        for h in range(HPC):
            r0 = (h % 2) * DH
            col = h // 2
            if h % 2 == 0:
                ss = [(QT[("x", col)], r0,
                       sqx_sb[r0:r0 + DH, col:col + 1], (QH[h], 0)),
                      (QT[("a", col)], r0,
                       sqa_sb[r0:r0 + DH, col:col + 1], (QSW[h], 0))]
            else:
                ss = [(QT[("x", col)], r0,
                       sqx_sb[r0:r0 + DH, col:col + 1], (QSW[h], DH)),
                      (QT[("a", col)], r0,
                       sqa_sb[r0:r0 + DH, col:col + 1], (QH[h], DH))]
            units.append((f"h{h}", QH[h], QSW[h], ss))

        qrot = [None] * HPC
        krot = KH
        with tc.tile_pool(name="nm", bufs=1, space="PSUM") as nm:
            for uname, ht, swt, ss in units:
                for src, r0, sc, (dst, dr0) in ss:
                    r1 = r0 + DH
                    q2 = sb.tile([P, n], BF16, tag="sq", bufs=3)
                    nc.vector.tensor_mul(q2[r0:r1, :], src[r0:r1, :],
                                         src[r0:r1, :])
                    psr = nm.tile([P, n], F32, tag="r", bufs=2, name=f"r_{uname}")
                    prc = sb.tile([P, n], F32, tag="prc", bufs=2)
                    for c in range(NCH):
                        cs = slice(c * CH, (c + 1) * CH)
                        nc.tensor.matmul(psr[r0:r1, cs], ones[r0:r1, 0:DH],
                                         q2[r0:r1, cs], start=True, stop=True)
                    if r0 == 0:
                        # sqrt in PSUM, then fast approx reciprocal (only
                        # correct at base partition 0 on HW)
                        nc.scalar.activation(psr[r0:r1, :], psr[r0:r1, :],
                                             AF.Sqrt, bias=eps_sb[r0:r1, :],
                                             scale=1.0)
                        nc.vector.reciprocal_approx_fast(out=prc[r0:r1, :],
                                                         in_=psr[r0:r1, :])
                    else:
                        # rsqrt = exp(-0.5 * ln(ss)) on the scalar engine
                        nc.scalar.activation(psr[r0:r1, :], psr[r0:r1, :],
                                             AF.Ln, bias=eps_sb[r0:r1, :],
                                             scale=1.0)
                        nc.scalar.activation(prc[r0:r1, :], psr[r0:r1, :],
                                             AF.Exp, bias=0.0, scale=-0.5)
                    nc.vector.scalar_tensor_tensor(
                        dst[dr0:dr0 + DH, :], src[r0:r1, :],
                        sc, prc[r0:r1, :], op0=ALU.mult, op1=ALU.mult)
                # companion-half swap DMAs (chunk-split across queues), then
                # rotary for this unit: rot(t) = t*cos + t_halfswap*sin_signed
                # (sin_sb rows 0:64 = -sin64, rows 64:128 = +sin64).
                # Even units write the upper halves directly (swap fills the
                # lower); odd heads are the mirror image.
                upper_direct = ss[0][3][1] == 0
                for c in range(NCH):
                    cs = slice(c * CH, (c + 1) * CH)
                    if upper_direct:
                        nc.sync.dma_start(out=swt[DH:P, cs], in_=ht[0:DH, cs])
                        nc.sync.dma_start(out=ht[DH:P, cs], in_=swt[0:DH, cs])
                    else:
                        nc.sync.dma_start(out=ht[0:DH, cs], in_=swt[DH:P, cs])
                        nc.sync.dma_start(out=swt[0:DH, cs], in_=ht[DH:P, cs])
                tcos = sb.tile([P, n], BF16, tag="tcos", bufs=1)
                tsin = sb.tile([P, n], BF16, tag="tsin", bufs=1)
                nc.vector.tensor_mul(tcos, ht, cos_sb)
                nc.vector.tensor_mul(tsin, swt, sin_sb)
                nc.vector.tensor_add(ht, tcos, tsin)
                if uname != "k":
                    qrot[int(uname[1:])] = ht

        if stage == 1:
            # dump projections + V
            nc.sync.dma_start(out=out[0], in_=QT[("x", 0)])
            nc.sync.dma_start(out=out[1], in_=QT[("a", 0)])
            nc.sync.dma_start(out=out[2], in_=KVX)
            for jt in range(NJT):
                nc.sync.dma_start(out=out[3][:, jt * P:(jt + 1) * P], in_=V[jt])
        elif stage == 2:
            # dump rotary q0/q1, krot, V
            nc.sync.dma_start(out=out[0], in_=qrot[0])
            nc.sync.dma_start(out=out[1], in_=qrot[1])
            nc.sync.dma_start(out=out[2], in_=krot)
            for jt in range(NJT):
                nc.sync.dma_start(out=out[3][:, jt * P:(jt + 1) * P], in_=V[jt])

        # ---------------- attention ----------------
        with tc.tile_pool(name="at", bufs=1, space="PSUM") as at:
          if stage == 0:
            def emit_scores(h, su, jt):
                js = slice(jt * P, (jt + 1) * P)
                ps_s = at.tile([P, SU], F32, tag="s", bufs=2, name=f"s{h}_{su}_{jt}")
                for cc in range(SUC):
                    el = slice(cc * CH, (cc + 1) * CH)
                    il = slice(su * SU + cc * CH, su * SU + (cc + 1) * CH)
                    nc.tensor.matmul(ps_s[:, el], krot[:, js], qrot[h][:, il],
                                     start=True, stop=True)
                return ps_s

            # software pipeline: scores(jt+1) is emitted (PE queue) before the
            # exp-dependent ones/AV matmuls of jt, so the PE never waits on
            # the scalar engine's exp round-trip.
            hsu = [(h, su) for h in range(HPC) for su in range(NSU)]
            for h, su in hsu:
                ps_o = at.tile([P, SU], F32, tag="o", bufs=1, name=f"o{h}_{su}")
                ps_e = at.tile([P, SU], F32, tag="e", bufs=1, name=f"e{h}_{su}")
                ps_s = emit_scores(h, su, 0)
                for jt in range(NJT):
                    es = sb.tile([P, SU], BF16, tag="es", bufs=3)
                    nc.scalar.activation(es, ps_s, AF.Exp, bias=0.0,
                                         scale=SM_SCALE)
                    if jt + 1 < NJT:
                        ps_s = emit_scores(h, su, jt + 1)
                    for cc in range(SUC):
                        el = slice(cc * CH, (cc + 1) * CH)
                        nc.tensor.matmul(ps_e[:, el], ones, es[:, el],
                                         start=(jt == 0), stop=(jt == NJT - 1))
                    for cc in range(SUC):
                        el = slice(cc * CH, (cc + 1) * CH)
                        nc.tensor.matmul(ps_o[:, el], V[jt], es[:, el],
                                         start=(jt == 0), stop=(jt == NJT - 1))
                rec = sb.tile([P, SU], F32, tag="rec", bufs=2)
                nc.vector.reciprocal_approx_fast(out=rec, in_=ps_e)
                on = sb.tile([P, SU], BF16, tag="on", bufs=2)
                nc.vector.tensor_mul(on, ps_o, rec)
                for cc in range(SUC):
                    el = slice(cc * CH, (cc + 1) * CH)
                    nc.sync.dma_start(
                        out=out[h, :, su * SU + cc * CH:su * SU + (cc + 1) * CH],
                        in_=on[:, el])
    nc.finalize()
    return nc


# ---------------------------------------------------------------------------
# host side
# ---------------------------------------------------------------------------

_NC_CACHE = {}


def get_nc(n=N, nb=B):
    key = n
    if key not in _NC_CACHE:
        _NC_CACHE[key] = build_nc(n)
    return _NC_CACHE[key]


def rotary_tables(n):
    inv_freq = 1.0 / (10000.0 ** (np.arange(0, ROT, 2, dtype=np.float64) / ROT))
    freqs = np.outer(np.arange(n, dtype=np.float64), inv_freq)  # [n, 64]
    cos64 = np.cos(freqs).T.astype(np.float32)                  # [64, n]
    sin64 = np.sin(freqs).T.astype(np.float32)
    cosT = np.ascontiguousarray(np.concatenate([cos64, cos64], 0)).astype(NPBF)
    sinT = np.ascontiguousarray(np.concatenate([-sin64, sin64], 0)).astype(NPBF)
    return cosT, sinT


def prep_in_maps(inputs, n=N, nb=B, ncores=NCORES):
    g = {k: np.asarray(v, dtype=np.float32) for k, v in inputs.items()}
    xT = [np.ascontiguousarray(g["x"][b].T).astype(NPBF) for b in range(nb)]
    aT = [np.ascontiguousarray(g["a"][b].T).astype(NPBF) for b in range(nb)]
    wkvx = np.ascontiguousarray(g["Wkv_x"].T).astype(NPBF)          # cols [kx|vx]
    wkva = np.ascontiguousarray(g["Wkv_a"].T).astype(NPBF)          # cols [ka|va]
    sk = np.zeros((P, 2), np.float32)                               # rows 0:64 only
    sk[0:DH, 0] = g["kx_scale"][0, 0]
    sk[0:DH, 1] = g["ka_scale"][0, 0]
    cosT, sinT = rotary_tables(n)

    in_maps = []
    for c in range(ncores):
        b = c // (ncores // nb)
        h0 = (c % (ncores // nb)) * HPC
        m = dict(xT=xT[b], aT=aT[b], wkvx=wkvx, wkva=wkva, sk=sk,
                 cosT=cosT, sinT=sinT)
        m["wqx"] = np.ascontiguousarray(
            g["Wq_x"][h0 * DH:(h0 + HPC) * DH].T).astype(NPBF)
        m["wqa"] = np.ascontiguousarray(
            g["Wq_a"][h0 * DH:(h0 + HPC) * DH].T).astype(NPBF)
        m["sqx"] = np.ascontiguousarray(np.stack(
            [np.concatenate([g["qx_scale"][h0 + 2 * t, 0],
                             g["qx_scale"][h0 + 2 * t + 1, 0]]) for t in range(2)],
            axis=1)).astype(np.float32)
        m["sqa"] = np.ascontiguousarray(np.stack(
            [np.concatenate([g["qa_scale"][h0 + 2 * t, 0],
                             g["qa_scale"][h0 + 2 * t + 1, 0]]) for t in range(2)],
            axis=1)).astype(np.float32)
        in_maps.append(m)
    return in_maps


def gather_out(results, n=N, nb=B, ncores=NCORES):
    full = np.empty((nb, n, HEADS * ROT), np.float32)
    for c in range(ncores):
        b = c // (ncores // nb)
        h0 = (c % (ncores // nb)) * HPC
        o = np.asarray(results[c]["out"]).astype(np.float32)  # [HPC, ROT, n]
        for h in range(HPC):
            gh = h0 + h
            full[b, :, gh * ROT:(gh + 1) * ROT] = o[h].T
    return full


def kernel(**inputs):
    from concourse.bass_utils import run_bass_kernel_spmd
    nc = get_nc(N, B)
    in_maps = prep_in_maps(inputs, N, B, NCORES)
    res = run_bass_kernel_spmd(nc, in_maps, list(range(NCORES)))
    return gather_out(res.results, N, B, NCORES)


if __name__ == "__main__":
    build_nc(256)
    print("build ok")


# revision 27
# speedup vs baseline: 3.6606x; 1.0045x over previous
"""CMAttention Trainium2 kernel (8-core SPMD, bf16 compute).

Reference computation (per nn_CMAttention):
  q_x = (x @ Wq_x.T)  -> [b, 16, n, 64],  q_a likewise
  kv_x = x @ Wkv_x.T -> k_x, v_x [b, 1, n, 64] (single shared KV head), kv_a likewise
  l2norm + learned scales on q_x/q_a (per head) and k_x/k_a (shared)
  q = concat(q_x, q_a) [b,16,n,128]; k, v likewise [b,1,n,128]
  rotary(q, k) over the 128-dim concat axis; SDPA with softmax over keys.

Sharding: each core owns ONE batch (core//4) and FOUR heads ((core%4)*4 ..).
The shared KV projection is computed replicated on the 4 cores of a batch.

Device-side layout: everything is computed "transposed" (feature dim on
partitions, sequence on the free axis). All matmuls run in bf16 (fp32 matmul
is 4 cycles/column on TRN2; bf16 is 1), accumulating in fp32 PSUM. Softmax
runs on S^T tiles: the row-sum over keys is a ones-matmul partition reduction
accumulated in PSUM; no max subtraction is needed because q/k rows are
l2-normalized (|scores*scale| <= ~0.2 for unit scales).

Per-head rotary layout trick: the qk-norm scalar_tensor_tensor writes its
output DIRECTLY into the per-head [x-half; a-half] rotary tiles wherever the
partition ranges line up (DVE ops need matching start partitions); the
mismatched half of each tile is filled with one SBUF->SBUF DMA from its
companion tile.
"""

import numpy as np
import ml_dtypes
from contextlib import ExitStack

import concourse.bass as bass
from concourse import bacc
import concourse.mybir as mybir
import concourse.tile as tile
from concourse.masks import make_identity

F32 = mybir.dt.float32
BF16 = mybir.dt.bfloat16
AF = mybir.ActivationFunctionType
ALU = mybir.AluOpType
NPBF = ml_dtypes.bfloat16

P = 128
B, N, DIM = 2, 2048, 1024
HEADS, DH, ROT = 16, 64, 128
NCORES = 8
HPC = 4                     # heads per core (one batch per core)
KT = DIM // P               # 8 contraction tiles
SM_SCALE = float(1.0 / np.sqrt(ROT))


def build_nc(n=N, stage=0):
    CH = min(512, n)        # fp32 PSUM bank = 512 floats
    NCH = n // CH
    SU = min(1024, n)       # attention superunit width (2 PSUM banks)
    NSU = n // SU
    SUC = SU // CH
    NJT = n // P            # key tiles

    nc = bacc.Bacc()
    dp = nc.declare_dram_parameter
    xT = dp("xT", [DIM, n], BF16, isOutput=False)
    aT = dp("aT", [DIM, n], BF16, isOutput=False)
    wqx = dp("wqx", [DIM, HPC * DH], BF16, isOutput=False)
    wqa = dp("wqa", [DIM, HPC * DH], BF16, isOutput=False)
    wkvx = dp("wkvx", [DIM, P], BF16, isOutput=False)  # cols [k_x | v_x]
    wkva = dp("wkva", [DIM, P], BF16, isOutput=False)  # cols [k_a | v_a]
    sqx = dp("sqx", [P, 2], F32, isOutput=False)       # col t: heads (2t, 2t+1)
    sqa = dp("sqa", [P, 2], F32, isOutput=False)
    sk = dp("sk", [P, 2], F32, isOutput=False)         # rows 0:64: col0 kx, col1 ka
    cosT = dp("cosT", [P, n], BF16, isOutput=False)    # [cos64; cos64]
    sinT = dp("sinT", [P, n], BF16, isOutput=False)    # [-sin64; sin64]
    out = dp("out", [HPC, ROT, n], BF16, isOutput=True)

    with ExitStack() as ctx:
        tc = ctx.enter_context(tile.TileContext(nc))
        consts = ctx.enter_context(tc.tile_pool(name="consts", bufs=1))
        sb = ctx.enter_context(tc.tile_pool(name="sb", bufs=1))

        ones = consts.tile([P, P], BF16)
        nc.vector.memset(ones, 1.0)
        eps_sb = consts.tile([P, 1], F32)
        nc.vector.memset(eps_sb, 1e-24)
        ident = consts.tile([P, P], BF16)
        make_identity(nc, ident)

        sqx_sb = consts.tile([P, 2], F32)
        nc.gpsimd.dma_start(out=sqx_sb, in_=sqx[:])
        sqa_sb = consts.tile([P, 2], F32)
        nc.gpsimd.dma_start(out=sqa_sb, in_=sqa[:])
        sk_sb = consts.tile([P, 2], F32)
        nc.gpsimd.dma_start(out=sk_sb, in_=sk[:])
        cos_sb = consts.tile([P, n], BF16)
        nc.sync.dma_start(out=cos_sb, in_=cosT[:])
        sin_sb = consts.tile([P, n], BF16)
        nc.sync.dma_start(out=sin_sb, in_=sinT[:])

        w_sb = {}
        for name, hdl, m in (("wqx", wqx, HPC * DH), ("wqa", wqa, HPC * DH),
                             ("wkvx", wkvx, P), ("wkva", wkva, P)):
            w_sb[name] = consts.tile([P, KT, m], BF16, name=f"w_{name}")
        for ki in range(KT):
            for name, hdl in (("wqx", wqx), ("wqa", wqa),
                              ("wkvx", wkvx), ("wkva", wkva)):
                nc.sync.dma_start(out=w_sb[name][:, ki, :],
                                  in_=hdl[ki * P:(ki + 1) * P, :])

        # ---------------- projections ----------------
        # Per modality: Q1 (heads 0-1), Q2 (heads 2-3), KV; chunk-major so the
        # PSUM working set stays at 3 tags x 2 bufs = 6 banks.
        QT = {}   # (mod, half) -> [P, n] bf16, rows [hEven dims | hOdd dims]
        KVX = sb.tile([P, n], BF16, tag="kvx")
        KVA = sb.tile([P, n], BF16, tag="kva")
        # chunk-split input loads (c-major): chunk 0 of every k-tile lands
        # first, spread over the DMA queues, so chunk-major matmuls can start
        # after ~1/NCH of the input DMA instead of all of it
        ktiles = {}
        for mod, src in (("x", xT), ("a", aT)):
            for ki in range(KT):
                ktiles[(mod, ki)] = sb.tile([P, n], BF16, tag="ktile", bufs=10,
                                            name=f"kt_{mod}{ki}")
        for c in range(NCH):
            cs = slice(c * CH, (c + 1) * CH)
            for mod, src in (("x", xT), ("a", aT)):
                for ki in range(KT):
                    nc.sync.dma_start(out=ktiles[(mod, ki)][:, cs],
                                      in_=src[ki * P:(ki + 1) * P, cs])

        with tc.tile_pool(name="pj", bufs=1, space="PSUM") as pj:
            for mod, wq_name, wkv_name, kvdst in (
                ("x", "wqx", "wkvx", KVX), ("a", "wqa", "wkva", KVA),
            ):
                q1 = sb.tile([P, n], BF16, tag=f"q1{mod}")
                q2t = sb.tile([P, n], BF16, tag=f"q2{mod}")
                QT[(mod, 0)] = q1
                QT[(mod, 1)] = q2t
                wq_t = w_sb[wq_name]
                wkv_t = w_sb[wkv_name]
                for c in range(NCH):
                    cs = slice(c * CH, (c + 1) * CH)
                    ps1 = pj.tile([P, CH], F32, tag="p1", bufs=2)
                    ps2 = pj.tile([P, CH], F32, tag="p2", bufs=2)
                    psk = pj.tile([P, CH], F32, tag="pk", bufs=2)
                    for ki in range(KT):
                        mv = ktiles[(mod, ki)][:, cs]
                        st = (ki == 0)
                        sp = (ki == KT - 1)
                        nc.tensor.matmul(ps1, wq_t[:, ki, 0:P], mv, start=st, stop=sp)
                        nc.tensor.matmul(ps2, wq_t[:, ki, P:2 * P], mv, start=st, stop=sp)
                        nc.tensor.matmul(psk, wkv_t[:, ki, :], mv, start=st, stop=sp)
                    nc.vector.tensor_copy(q1[:, cs], ps1)
                    nc.vector.tensor_copy(q2t[:, cs], ps2)
                    nc.vector.tensor_copy(kvdst[:, cs], psk)

        # ---------------- V transpose ----------------
        # V_jt [j, d]: cols 0:64 = v_x (KVX rows 64:128), cols 64:128 = v_a
        # (KVA rows 64:128)
        V = []
        with tc.tile_pool(name="vt", bufs=2, space="PSUM") as vtp:
            for jt in range(NJT):
                js = slice(jt * P, (jt + 1) * P)
                psv1 = vtp.tile([P, DH], BF16, tag="v1")
                psv2 = vtp.tile([P, DH], BF16, tag="v2")
                nc.tensor.transpose(psv1, KVX[DH:P, js], ident[DH:P, DH:P])
                nc.tensor.transpose(psv2, KVA[DH:P, js], ident[DH:P, DH:P])
                vj = sb.tile([P, P], BF16, tag="vsb", bufs=NJT)
                nc.vector.tensor_copy(vj[:, 0:DH], psv1)
                nc.vector.tensor_copy(vj[:, DH:P], psv2)
                V.append(vj)

        # ---------------- qk-norm + per-head rotary layout ----------------
        # Per-head tiles: qh[h] rows [x-half; a-half], qsw[h] rows
        # [a-half; x-half]. The stt writes whichever target matches the source
        # partition range; the companion half is a SBUF->SBUF DMA copy.
        #
        # All streams are emitted batched by op type (squares -> sum matmuls
        # -> rsqrt -> stt -> swap DMAs) so the per-stream PE->ACT->DVE chains
        # pipeline across streams instead of serializing.
        QH = [sb.tile([P, n], BF16, tag=f"qh{h}", name=f"qh{h}") for h in range(HPC)]
        QSW = [sb.tile([P, n], BF16, tag=f"qsw{h}", name=f"qsw{h}") for h in range(HPC)]
        KH = sb.tile([P, n], BF16, tag="kh")
        KSW = sb.tile([P, n], BF16, tag="ksw")

        # units: K first (every head's attention needs krot), then heads in
        # order. Each stream: (src, r0, scale, (direct_dst, dst_r0)).
        units = [("k", KH, KSW,
                  [(KVX, 0, sk_sb[0:DH, 0:1], (KH, 0)),
                   (KVA, 0, sk_sb[0:DH, 1:2], (KSW, 0))])]
        for h in range(HPC):
            r0 = (h % 2) * DH
            col = h // 2
            if h % 2 == 0:
                ss = [(QT[("x", col)], r0,
                       sqx_sb[r0:r0 + DH, col:col + 1], (QH[h], 0)),
                      (QT[("a", col)], r0,
                       sqa_sb[r0:r0 + DH, col:col + 1], (QSW[h], 0))]
            else:
                ss = [(QT[("x", col)], r0,
                       sqx_sb[r0:r0 + DH, col:col + 1], (QSW[h], DH)),
                      (QT[("a", col)], r0,
                       sqa_sb[r0:r0 + DH, col:col + 1], (QH[h], DH))]
            units.append((f"h{h}", QH[h], QSW[h], ss))

        qrot = [None] * HPC
        krot = KH
        with tc.tile_pool(name="nm", bufs=1, space="PSUM") as nm:
            for uname, ht, swt, ss in units:
                for src, r0, sc, (dst, dr0) in ss:
                    r1 = r0 + DH
                    q2 = sb.tile([P, n], BF16, tag="sq", bufs=3)
                    nc.vector.tensor_mul(q2[r0:r1, :], src[r0:r1, :],
                                         src[r0:r1, :])
                    psr = nm.tile([P, n], F32, tag="r", bufs=2, name=f"r_{uname}")
                    prc = sb.tile([P, n], F32, tag="prc", bufs=2)
                    for c in range(NCH):
                        cs = slice(c * CH, (c + 1) * CH)
                        nc.tensor.matmul(psr[r0:r1, cs], ones[r0:r1, 0:DH],
                                         q2[r0:r1, cs], start=True, stop=True)
                    if r0 == 0:
                        # sqrt in PSUM, then fast approx reciprocal (only
                        # correct at base partition 0 on HW)
                        nc.scalar.activation(psr[r0:r1, :], psr[r0:r1, :],
                                             AF.Sqrt, bias=eps_sb[r0:r1, :],
                                             scale=1.0)
                        nc.vector.reciprocal_approx_fast(out=prc[r0:r1, :],
                                                         in_=psr[r0:r1, :])
                    else:
                        # rsqrt = exp(-0.5 * ln(ss)) on the scalar engine
                        nc.scalar.activation(psr[r0:r1, :], psr[r0:r1, :],
                                             AF.Ln, bias=eps_sb[r0:r1, :],
                                             scale=1.0)
                        nc.scalar.activation(prc[r0:r1, :], psr[r0:r1, :],
                                             AF.Exp, bias=0.0, scale=-0.5)
                    nc.vector.scalar_tensor_tensor(
                        dst[dr0:dr0 + DH, :], src[r0:r1, :],
                        sc, prc[r0:r1, :], op0=ALU.mult, op1=ALU.mult)
                # companion-half swap DMAs (chunk-split across queues), then
                # rotary for this unit: rot(t) = t*cos + t_halfswap*sin_signed
                # (sin_sb rows 0:64 = -sin64, rows 64:128 = +sin64).
                # Even units write the upper halves directly (swap fills the
                # lower); odd heads are the mirror image.
                upper_direct = ss[0][3][1] == 0
                for c in range(NCH):
                    cs = slice(c * CH, (c + 1) * CH)
                    if upper_direct:
                        nc.sync.dma_start(out=swt[DH:P, cs], in_=ht[0:DH, cs])
                        nc.sync.dma_start(out=ht[DH:P, cs], in_=swt[0:DH, cs])
                    else:
                        nc.sync.dma_start(out=ht[0:DH, cs], in_=swt[DH:P, cs])
                        nc.sync.dma_start(out=swt[0:DH, cs], in_=ht[DH:P, cs])
                tcos = sb.tile([P, n], BF16, tag="tcos", bufs=1)
                tsin = sb.tile([P, n], BF16, tag="tsin", bufs=1)
                nc.vector.tensor_mul(tcos, ht, cos_sb)
                nc.vector.tensor_mul(tsin, swt, sin_sb)
                nc.vector.tensor_add(ht, tcos, tsin)
                if uname != "k":
                    qrot[int(uname[1:])] = ht

        if stage == 1:
            # dump projections + V
            nc.sync.dma_start(out=out[0], in_=QT[("x", 0)])
            nc.sync.dma_start(out=out[1], in_=QT[("a", 0)])
            nc.sync.dma_start(out=out[2], in_=KVX)
            for jt in range(NJT):
                nc.sync.dma_start(out=out[3][:, jt * P:(jt + 1) * P], in_=V[jt])
        elif stage == 2:
            # dump rotary q0/q1, krot, V
            nc.sync.dma_start(out=out[0], in_=qrot[0])
            nc.sync.dma_start(out=out[1], in_=qrot[1])
            nc.sync.dma_start(out=out[2], in_=krot)
            for jt in range(NJT):
                nc.sync.dma_start(out=out[3][:, jt * P:(jt + 1) * P], in_=V[jt])

        # ---------------- attention ----------------
        with tc.tile_pool(name="at", bufs=1, space="PSUM") as at:
          if stage == 0:
            def emit_scores(h, su, jt):
                js = slice(jt * P, (jt + 1) * P)
                ps_s = at.tile([P, SU], F32, tag="s", bufs=2, name=f"s{h}_{su}_{jt}")
                for cc in range(SUC):
                    el = slice(cc * CH, (cc + 1) * CH)
                    il = slice(su * SU + cc * CH, su * SU + (cc + 1) * CH)
                    nc.tensor.matmul(ps_s[:, el], krot[:, js], qrot[h][:, il],
                                     start=True, stop=True)
                return ps_s

            # software pipeline: scores(jt+1) is emitted (PE queue) before the
            # exp-dependent AV matmuls of jt, so the PE never waits on the
            # scalar engine's exp round-trip. The softmax denominator is
            # accumulated across key-tiles on the vector engine (bf16 adds)
            # and partition-reduced with a single ones-matmul at the end,
            # instead of a per-key-tile ones-matmul on the PE.
            hsu = [(h, su) for h in range(HPC) for su in range(NSU)]
            for h, su in hsu:
                ps_o = at.tile([P, SU], F32, tag="o", bufs=1, name=f"o{h}_{su}")
                ps_s = emit_scores(h, su, 0)
                acc = None
                for jt in range(NJT):
                    es = sb.tile([P, SU], BF16, tag="es", bufs=3)
                    nc.scalar.activation(es, ps_s, AF.Exp, bias=0.0,
                                         scale=SM_SCALE)
                    if jt + 1 < NJT:
                        ps_s = emit_scores(h, su, jt + 1)
                    for cc in range(SUC):
                        el = slice(cc * CH, (cc + 1) * CH)
                        nc.tensor.matmul(ps_o[:, el], V[jt], es[:, el],
                                         start=(jt == 0), stop=(jt == NJT - 1))
                    if acc is None:
                        acc = es
                    else:
                        nacc = sb.tile([P, SU], BF16, tag="acc", bufs=2)
                        nc.vector.tensor_add(nacc, acc, es)
                        acc = nacc
                ps_den = at.tile([P, SU], F32, tag="s", bufs=2, name=f"d{h}_{su}")
                for cc in range(SUC):
                    el = slice(cc * CH, (cc + 1) * CH)
                    nc.tensor.matmul(ps_den[:, el], ones, acc[:, el],
                                     start=True, stop=True)
                rec = sb.tile([P, SU], F32, tag="rec", bufs=2)
                nc.vector.reciprocal_approx_fast(out=rec, in_=ps_den)
                on = sb.tile([P, SU], BF16, tag="on", bufs=2)
                nc.vector.tensor_mul(on, ps_o, rec)
                for cc in range(SUC):
                    el = slice(cc * CH, (cc + 1) * CH)
                    nc.sync.dma_start(
                        out=out[h, :, su * SU + cc * CH:su * SU + (cc + 1) * CH],
                        in_=on[:, el])
    nc.finalize()
    return nc


# ---------------------------------------------------------------------------
# host side
# ---------------------------------------------------------------------------

_NC_CACHE = {}


def get_nc(n=N, nb=B):
    key = n
    if key not in _NC_CACHE:
        _NC_CACHE[key] = build_nc(n)
    return _NC_CACHE[key]


def rotary_tables(n):
    inv_freq = 1.0 / (10000.0 ** (np.arange(0, ROT, 2, dtype=np.float64) / ROT))
    freqs = np.outer(np.arange(n, dtype=np.float64), inv_freq)  # [n, 64]
    cos64 = np.cos(freqs).T.astype(np.float32)                  # [64, n]
    sin64 = np.sin(freqs).T.astype(np.float32)
    cosT = np.ascontiguousarray(np.concatenate([cos64, cos64], 0)).astype(NPBF)
    sinT = np.ascontiguousarray(np.concatenate([-sin64, sin64], 0)).astype(NPBF)
    return cosT, sinT


def prep_in_maps(inputs, n=N, nb=B, ncores=NCORES):
    g = {k: np.asarray(v, dtype=np.float32) for k, v in inputs.items()}
    xT = [np.ascontiguousarray(g["x"][b].T).astype(NPBF) for b in range(nb)]
    aT = [np.ascontiguousarray(g["a"][b].T).astype(NPBF) for b in range(nb)]
    wkvx = np.ascontiguousarray(g["Wkv_x"].T).astype(NPBF)          # cols [kx|vx]
    wkva = np.ascontiguousarray(g["Wkv_a"].T).astype(NPBF)          # cols [ka|va]
    sk = np.zeros((P, 2), np.float32)                               # rows 0:64 only
    sk[0:DH, 0] = g["kx_scale"][0, 0]
    sk[0:DH, 1] = g["ka_scale"][0, 0]
    cosT, sinT = rotary_tables(n)

    in_maps = []
    for c in range(ncores):
        b = c // (ncores // nb)
        h0 = (c % (ncores // nb)) * HPC
        m = dict(xT=xT[b], aT=aT[b], wkvx=wkvx, wkva=wkva, sk=sk,
                 cosT=cosT, sinT=sinT)
        m["wqx"] = np.ascontiguousarray(
            g["Wq_x"][h0 * DH:(h0 + HPC) * DH].T).astype(NPBF)
        m["wqa"] = np.ascontiguousarray(
            g["Wq_a"][h0 * DH:(h0 + HPC) * DH].T).astype(NPBF)
        m["sqx"] = np.ascontiguousarray(np.stack(
            [np.concatenate([g["qx_scale"][h0 + 2 * t, 0],
                             g["qx_scale"][h0 + 2 * t + 1, 0]]) for t in range(2)],
            axis=1)).astype(np.float32)
        m["sqa"] = np.ascontiguousarray(np.stack(
            [np.concatenate([g["qa_scale"][h0 + 2 * t, 0],
                             g["qa_scale"][h0 + 2 * t + 1, 0]]) for t in range(2)],
            axis=1)).astype(np.float32)
        in_maps.append(m)
    return in_maps


def gather_out(results, n=N, nb=B, ncores=NCORES):
    full = np.empty((nb, n, HEADS * ROT), np.float32)
    for c in range(ncores):
        b = c // (ncores // nb)
        h0 = (c % (ncores // nb)) * HPC
        o = np.asarray(results[c]["out"]).astype(np.float32)  # [HPC, ROT, n]
        for h in range(HPC):
            gh = h0 + h
            full[b, :, gh * ROT:(gh + 1) * ROT] = o[h].T
    return full


def kernel(**inputs):
    from concourse.bass_utils import run_bass_kernel_spmd
    nc = get_nc(N, B)
    in_maps = prep_in_maps(inputs, N, B, NCORES)
    res = run_bass_kernel_spmd(nc, in_maps, list(range(NCORES)))
    return gather_out(res.results, N, B, NCORES)


if __name__ == "__main__":
    build_nc(256)
    print("build ok")
